# revision 14
# baseline (speedup 1.0000x reference)
"""Bass/Trainium2 kernel for nn_NodesToEdges (gnn_message_passing).

out[b,i,j,:] = rms(edges[b,i,j,:])*g_e @ We + rms(nodes[b,i,:])*g_n @ Wr
             + rms(nodes[b,j,:])*g_n @ Wc + bias

Strategy: shard over i (rows) across 8 cores. The tiny node path
(row_proj / col_proj) is precomputed on host. The edge path (134 MB in /
134 MB out, memory-bound) runs on device.

Per core: 128 blocks (b,i) of [512 j, 64 e], processed as 32 TILES of 4
blocks = [128 part = (a,p=j//16), 1024 free = (r=j%16, e)] (4 KB DMA
descriptors). Per tile:
  ACT: square -> DVE: 16-group reduce -> ACT: sqrt(mean+eps) ->
  DVE: recip -> Pool: prescale (x*inv -> bf16) ->
  PE: 8x transpose (128-col chunks, bf16) -> DVE: copy psum->sbuf ->
  PE: row/col adds via indicator-stationary matmuls + 8 chunk-matmuls
      with the TRANSPOSED DATA as stationary (output lands row-major,
      f32 psum; no back-transpose) ->
  ACT: copy psum->sbuf -> Pool: SWDGE out-DMA (4-tile batches).
In-DMA on the SP HWDGE ring; out-DMA on the Pool SWDGE ring.
"""

import numpy as np

B, N, DE, DN = 2, 512, 64, 128
NCORES = 8
IPC = N // NCORES          # 64 i-rows per core
NBLK = B * IPC             # 128 blocks of [512, 64] per core
NT = NBLK // 4             # 32 tiles of 4 blocks
EPS = float(np.finfo(np.float32).eps)

# buffer depths (slots)
DXI = 6    # xin tiles in flight
DQ = 3     # sq slots
DS = 3     # ss / rms / inv slots
DXS = 3    # xs (prescaled bf16)
DP1 = 4    # psum transpose banks ([128,512] f32); tile t uses pair 2*(t%2)+{0,1}, chunk c -> bank c%2
DXT = 3    # xTs (sbuf bf16)
DP2 = 2    # psum matmul banks (2 banks each)
DO = 8     # outsb slots (two 4-tile out-DMA batches)


def _build_nc(nt=NT):
    from contextlib import ExitStack

    import concourse.bass as bass
    import concourse.mybir as mybir

    f32 = mybir.dt.float32
    bf16 = mybir.dt.bfloat16
    SQRT = mybir.ActivationFunctionType.Sqrt

    nc = bass.Bass()
    nblk = 4 * nt
    x_d = nc.declare_dram_parameter("x", [nblk, N, DE], f32, isOutput=False)
    wg_d = nc.declare_dram_parameter("wg", [128, 128], bf16, isOutput=False)
    idb_d = nc.declare_dram_parameter("idb", [128, 128], bf16, isOutput=False)
    i4_d = nc.declare_dram_parameter("i4", [4, 128], bf16, isOutput=False)
    i32_d = nc.declare_dram_parameter("i32", [32, 128], bf16, isOutput=False)
    rpm_d = nc.declare_dram_parameter("rpm", [4, 64 * nt], bf16, isOutput=False)
    cpm0_d = nc.declare_dram_parameter("cpm0", [32, 1024], bf16, isOutput=False)
    cpm1_d = nc.declare_dram_parameter("cpm1", [32, 1024], bf16, isOutput=False)
    out_d = nc.declare_dram_parameter("out", [nblk, N, DE], f32, isOutput=True)

    st = ExitStack()
    with st:
        sb = lambda name, shape, dt: st.enter_context(nc.sbuf_tensor(name, shape, dt))
        wg = sb("wg_sb", [128, 128], bf16)
        idb = sb("idb_sb", [128, 128], bf16)
        i4 = sb("i4_sb", [4, 128], bf16)
        i32 = sb("i32_sb", [32, 128], bf16)
        rpm = sb("rpm_sb", [4, 64 * nt], bf16)
        cpm0 = sb("cpm0_sb", [32, 1024], bf16)
        cpm1 = sb("cpm1_sb", [32, 1024], bf16)
        epsb = sb("epsb", [128, 1], f32)
        xin = [sb(f"xin{i}", [128, 1024], f32) for i in range(DXI)]
        sq = [sb(f"sq{i}", [128, 1024], f32) for i in range(DQ)]
        ss = [sb(f"ss{i}", [128, 16], f32) for i in range(DS)]
        rms = [sb(f"rms{i}", [128, 16], f32) for i in range(DS)]
        inv = [sb(f"inv{i}", [128, 16], f32) for i in range(DS)]
        xs = [sb(f"xs{i}", [128, 1024], bf16) for i in range(DXS)]
        xTs = [sb(f"xTs{i}", [128, 1024], bf16) for i in range(DXT)]
        outsb = sb("outsb", [128, DO * 1024], f32)
        ps1 = [
            st.enter_context(nc.psum_tensor(f"ps1{i}", [128, 512], f32))
            for i in range(DP1)
        ]
        ps2 = [
            st.enter_context(nc.psum_tensor(f"ps2{i}", [128, 1024], f32))
            for i in range(DP2)
        ]

        sem = lambda name: st.enter_context(nc.semaphore(name))
        s_c = sem("s_c")
        s_in = [sem(f"s_in{i}") for i in range(DXI)]
        s_od = [sem(f"s_od{i}") for i in range(2)]
        s_act = sem("s_act")
        s_dve = sem("s_dve")
        s_pl = sem("s_pl")
        s_pe = sem("s_pe")

        # --- precomputed semaphore values: done[(op, t)] = sem value once
        # op(t) has retired.
        done = {}
        for t in range(nt):
            done[("in", t)] = 16 * (t // DXI + 1)      # on s_in[t % DXI]
        ng = nt // 4
        for g in range(ng):
            done[("outd", g)] = 16 * (g // 2 + 1)      # on s_od[g % 2]

        # ACT iter u: final(u-9), square(u-1), sqrt(u-3)  (final FIRST so
        # PE's ps2-slot wait resolves a full iteration early)
        c = 0
        for u in range(nt + 11):
            if 0 <= u - 9 < nt:
                c += 1
                done[("final", u - 9)] = c
            if 0 <= u - 1 < nt:
                c += 1
                done[("square", u - 1)] = c
            if 0 <= u - 3 < nt:
                c += 1
                done[("sqrt", u - 3)] = c
        # DVE iter u: reduce(u-2), recip(u-4), copy1(u-7)
        c = 0
        for u in range(nt + 8):
            if 0 <= u - 2 < nt:
                c += 1
                done[("reduce", u - 2)] = c
            if 0 <= u - 4 < nt:
                c += 1
                done[("recip", u - 4)] = c
            if 0 <= u - 7 < nt:
                c += 1
                done[("c1a", u - 7)] = c
                c += 1
                done[("copy1", u - 7)] = c
        # Pool iter u: prescale(u-5)
        c = 0
        for u in range(nt + 6):
            if 0 <= u - 5 < nt:
                c += 1
                done[("prescale", u - 5)] = c
        # PE iter u: fwdT(u-6) x8 normal-mode identity matmuls, then
        # row/col + chunk-mms(u-8) x12. ONE s_pe inc per iteration on the
        # very last matmul -- mid-stream sem updates stall the next matmul
        # until the updating op retires, so expose none.
        c = 0
        for u in range(nt + 9):
            k = 0
            if 0 <= u - 6 < nt:
                k += 8
            if 0 <= u - 8 < nt:
                k += 12
            c += k
            if 0 <= u - 6 < nt:
                done[("fwdT", u - 6)] = c
            if 0 <= u - 8 < nt:
                done[("mms", u - 8)] = c
            done[("peiter", u)] = k

        CONST_TARGET = 7 * 16

        def in_src(t):
            return x_d[4 * t : 4 * t + 4].rearrange(
                "a (p r) e -> (a p) (r e)", p=32, r=16
            )

        def out_dst(g):
            return out_d[16 * g : 16 * g + 16].rearrange(
                "(T a) (p r) e -> (a p) T (r e)", T=4, a=4, p=32, r=16
            )

        def g16(x):
            return x.rearrange("p (g e) -> p g e", e=DE)

        with nc.Block() as block:

            @block.sync
            def _(sync):
                for cdst, csrc in (
                    (wg, wg_d),
                    (idb, idb_d),
                    (i4, i4_d),
                    (i32, i32_d),
                    (rpm, rpm_d),
                    (cpm0, cpm0_d),
                    (cpm1, cpm1_d),
                ):
                    sync.dma_start(out=cdst[:], in_=csrc[:]).then_inc(s_c, 16)
                for t in range(nt):
                    if t >= DXI:
                        sync.wait_ge(s_pl, done[("prescale", t - DXI)])
                    sync.dma_start(out=xin[t % DXI][:], in_=in_src(t)).then_inc(
                        s_in[t % DXI], 16
                    )

            @block.scalar
            def _(scalar):
                scalar.wait_ge(s_c, CONST_TARGET)
                for u in range(nt + 11):
                    if 0 <= u - 9 < nt:
                        t = u - 9
                        scalar.wait_ge(s_pe, done[("mms", t)])
                        if t >= DO:
                            gp = (t - DO) // 4
                            scalar.wait_ge(s_od[gp % 2], done[("outd", gp)])
                        nc.scalar.copy(
                            outsb[:, 1024 * (t % DO) : 1024 * (t % DO) + 1024],
                            ps2[t % DP2][:],
                        ).then_inc(s_act, 1)
                    if 0 <= u - 1 < nt:
                        t = u - 1
                        scalar.wait_ge(s_in[t % DXI], done[("in", t)])
                        if t >= DQ:
                            scalar.wait_ge(s_dve, done[("reduce", t - DQ)])
                        nc.scalar.square(sq[t % DQ][:], xin[t % DXI][:]).then_inc(
                            s_act, 1
                        )
                    if 0 <= u - 3 < nt:
                        t = u - 3
                        scalar.wait_ge(s_dve, done[("reduce", t)])
                        if t >= DS:
                            scalar.wait_ge(s_dve, done[("recip", t - DS)])
                        nc.scalar.activation(
                            rms[t % DS][:], ss[t % DS][:], SQRT,
                            bias=epsb[:], scale=1.0 / DE,
                        ).then_inc(s_act, 1)

            @block.vector
            def _(vector):
                vector.wait_ge(s_c, CONST_TARGET)
                nc.vector.memset(epsb[:], EPS)
                for u in range(nt + 8):
                    if 0 <= u - 2 < nt:
                        t = u - 2
                        vector.wait_ge(s_act, done[("square", t)])
                        if t >= DS:
                            vector.wait_ge(s_act, done[("sqrt", t - DS)])
                        nc.vector.tensor_reduce(
                            ss[t % DS][:],
                            g16(sq[t % DQ][:]),
                            axis=mybir.AxisListType.X,
                            op=mybir.AluOpType.add,
                        ).then_inc(s_dve, 1)
                    if 0 <= u - 4 < nt:
                        t = u - 4
                        vector.wait_ge(s_act, done[("sqrt", t)])
                        if t >= DS:
                            vector.wait_ge(s_pl, done[("prescale", t - DS)])
                        nc.vector.reciprocal(inv[t % DS][:], rms[t % DS][:]).then_inc(
                            s_dve, 1
                        )
                    if 0 <= u - 7 < nt:
                        t = u - 7
                        vector.wait_ge(s_pe, done[("fwdT", t)])
                        if t >= DXT:
                            vector.wait_ge(s_pe, done[("mms", t - DXT)])
                        xv = xTs[t % DXT][:].rearrange(
                            "p (c2 o e) -> p c2 o e", o=2, e=128
                        )
                        for o in range(2):
                            nc.vector.tensor_copy(
                                xv[:, :, o],
                                ps1[2 * (t % 2) + o][:].rearrange(
                                    "p (c2 e) -> p c2 e", e=128
                                ),
                            ).then_inc(s_dve, 1)

            @block.gpsimd
            def _(pool):
                pool.wait_ge(s_c, CONST_TARGET)
                for u in range(nt + 15):
                    if 0 <= u - 5 < nt:
                        t = u - 5
                        pool.wait_ge(s_dve, done[("recip", t)])
                        if t >= DXS:
                            pool.wait_ge(s_pe, done[("fwdT", t - DXS)])
                        nc.gpsimd.tensor_mul(
                            g16(xs[t % DXS][:]),
                            g16(xin[t % DXI][:]),
                            inv[t % DS][:].unsqueeze(-1).broadcast_to([128, 16, DE]),
                        ).then_inc(s_pl, 1)
                    if u >= 13 and (u - 13) % 4 == 0 and (u - 13) // 4 < ng:
                        g = (u - 13) // 4
                        pool.wait_ge(s_act, done[("final", 4 * g + 3)])
                        half = 4096 * ((4 * g) % DO // 4)
                        pool.dma_start(
                            out=out_dst(g), in_=outsb[:, half : half + 4096]
                        ).then_inc(s_od[g % 2], 16)

            @block.tensor
            def _(tensor):
                tensor.wait_ge(s_c, CONST_TARGET)
                for u in range(nt + 9):
                    tT = u - 6
                    tM = u - 8
                    doT = 0 <= tT < nt
                    doM = 0 <= tM < nt
                    inc = done[("peiter", u)]
                    if doT:
                        tensor.wait_ge(s_pl, done[("prescale", tT)])
                        if tT >= 2:
                            tensor.wait_ge(s_dve, done[("copy1", tT - 2)])
                        for c in range(8):
                            mi = nc.tensor.matmul(
                                ps1[2 * (tT % 2) + c % 2][
                                    :, 128 * (c // 2) : 128 * (c // 2) + 128
                                ],
                                xs[tT % DXS][:, 128 * c : 128 * c + 128],
                                idb[:],
                                start=True, stop=True, skip_group_check=True,
                            )
                            if c == 7 and not doM:
                                mi.then_inc(s_pe, inc)
                    if doM:
                        if tM >= DP2:
                            tensor.wait_ge(s_act, done[("final", tM - DP2)])
                        pt = ps2[tM % DP2]
                        rp = (
                            rpm[:, 64 * tM : 64 * tM + 64]
                            .unsqueeze(1)
                            .broadcast_to([4, 8, DE])
                        )
                        cpm = cpm0 if tM < nt // 2 else cpm1
                        for h in range(2):
                            nc.tensor.matmul(
                                pt[:, 512 * h : 512 * h + 512], i4[:], rp,
                                start=True, stop=False, skip_group_check=True,
                            )
                        for h in range(2):
                            nc.tensor.matmul(
                                pt[:, 512 * h : 512 * h + 512],
                                i32[:],
                                cpm[:, 512 * h : 512 * h + 512],
                                start=False, stop=False, skip_group_check=True,
                            )
                        tensor.wait_ge(s_dve, done[("copy1", tM)])
                        for c in range(8):
                            mi = nc.tensor.matmul(
                                pt[:, 128 * c : 128 * c + 128],
                                xTs[tM % DXT][:, 128 * c : 128 * c + 128],
                                wg[:],
                                start=False, stop=True, skip_group_check=True,
                            )
                            if c == 7:
                                mi.then_inc(s_pe, inc)

    return nc


_NC_CACHE = {}


def _get_nc():
    if "nc" not in _NC_CACHE:
        _NC_CACHE["nc"] = _build_nc()
    return _NC_CACHE["nc"]


def _make_in_maps(edges, nodes, g_node, g_edge, W, b):
    import ml_dtypes

    bf = ml_dtypes.bfloat16
    edges = np.ascontiguousarray(edges, dtype=np.float32)
    nodes = np.ascontiguousarray(nodes, dtype=np.float32)

    # ---- host: tiny node path (B*N*dn = 131K elems)
    ms = np.mean(np.square(nodes), axis=-1, keepdims=True)
    nodes_n = nodes / np.sqrt(ms + EPS) * g_node  # [B, N, 128]
    Wr, Wc, We = W[:DN], W[DN : 2 * DN], W[2 * DN :]
    row_proj = (nodes_n @ Wr).astype(np.float32)  # [B, N, 64]
    col_proj = (nodes_n @ Wc).astype(np.float32)  # [B, N, 64]
    Wg = (g_edge[:, None] * We).astype(np.float32)  # fold g_edge into We

    # block-diagonal Wg (even-r rows top-left, odd-r rows bottom-right)
    wgblk = np.zeros((128, 128), dtype=np.float32)
    wgblk[:64, :64] = Wg
    wgblk[64:, 64:] = Wg
    wgblk = wgblk.astype(bf)
    idb = np.eye(128, dtype=np.float32).astype(bf)
    i4 = np.kron(np.eye(4, dtype=np.float32), np.ones((1, 32), np.float32)).astype(bf)
    i32 = np.tile(np.eye(32, dtype=np.float32), (1, 4)).astype(bf)

    # col_proj + bias, per b: cpm_b[p', 64r + e'] = cp_b[16 p' + r, e']
    cp = (col_proj + b).astype(np.float32)  # [B, 512, 64]
    cpm0 = np.ascontiguousarray(cp[0].reshape(32, 1024)).astype(bf)
    cpm1 = np.ascontiguousarray(cp[1].reshape(32, 1024)).astype(bf)

    in_maps = []
    for c in range(NCORES):
        xc = edges[:, c * IPC : (c + 1) * IPC]  # [B, 64, 512, 64]
        xc = np.ascontiguousarray(xc).reshape(NBLK, N, DE)
        # rpm[k, 64T + e'] = row_proj[block 4T + k][e']
        rp = row_proj[:, c * IPC : (c + 1) * IPC].reshape(NBLK, DE)  # [128, 64]
        rpm = np.ascontiguousarray(
            rp.reshape(NT, 4, DE).transpose(1, 0, 2).reshape(4, NT * DE)
        ).astype(bf)
        in_maps.append(
            {
                "x": xc,
                "wg": wgblk,
                "idb": idb,
                "i4": i4,
                "i32": i32,
                "rpm": rpm,
                "cpm0": cpm0,
                "cpm1": cpm1,
            }
        )
    return in_maps


def _run(edges, nodes, g_node, g_edge, W, b, trace=False, **spmd_kwargs):
    in_maps = _make_in_maps(edges, nodes, g_node, g_edge, W, b)

    from concourse.bass_utils import run_bass_kernel_spmd

    nc = _get_nc()
    res = run_bass_kernel_spmd(
        nc, in_maps, list(range(NCORES)), trace=trace, **spmd_kwargs
    )

    out = np.empty((B, N, N, DE), dtype=np.float32)
    for c in range(NCORES):
        oc = res.results[c]["out"].reshape(B, IPC, N, DE)
        out[:, c * IPC : (c + 1) * IPC] = oc
    return out, res


def kernel(edges, nodes, g_node, g_edge, W, b):
    return _run(edges, nodes, g_node, g_edge, W, b)[0]


if __name__ == "__main__":
    rng = np.random.default_rng(0)
    edges = rng.standard_normal((B, N, N, DE), dtype=np.float32)
    nodes = rng.standard_normal((B, N, DN), dtype=np.float32)
    g_node = np.ones(DN, np.float32)
    g_edge = np.ones(DE, np.float32)
    W = rng.standard_normal((2 * DN + DE, DE), dtype=np.float32) / 18.0
    b = (rng.standard_normal(DE) * 0.01).astype(np.float32)
    o = kernel(edges, nodes, g_node, g_edge, W, b)
    print(o.shape, o.dtype)


# revision 15
# speedup vs baseline: 1.1603x; 1.1603x over previous
"""Bass/Trainium2 kernel for nn_NodesToEdges (gnn_message_passing).

out[b,i,j,:] = rms(edges[b,i,j,:])*g_e @ We + rms(nodes[b,i,:])*g_n @ Wr
             + rms(nodes[b,j,:])*g_n @ Wc + bias

Strategy: shard over i (rows) across 8 cores. The tiny node path
(row_proj / col_proj) is precomputed on host. The edge path (134 MB in /
134 MB out, memory-bound) runs on device.

Per core: 128 blocks (b,i) of [512 j, 64 e], processed as 32 TILES of 4
blocks = [128 part = (a,p=j//16), 1024 free = (r=j%16, e)] (4 KB DMA
descriptors). Per tile:
  ACT: square -> DVE: 16-group reduce -> ACT: sqrt(mean+eps) ->
  DVE: recip -> Pool: prescale (x*inv -> bf16) ->
  PE: 8x transpose (128-col chunks, bf16) -> DVE: copy psum->sbuf ->
  PE: row/col adds via indicator-stationary matmuls + 8 chunk-matmuls
      with the TRANSPOSED DATA as stationary (output lands row-major,
      f32 psum; no back-transpose) ->
  ACT: copy psum->sbuf -> Pool: SWDGE out-DMA (4-tile batches).
In-DMA on the SP HWDGE ring; out-DMA on the Pool SWDGE ring.
"""

import numpy as np

B, N, DE, DN = 2, 512, 64, 128
NCORES = 8
IPC = N // NCORES          # 64 i-rows per core
NBLK = B * IPC             # 128 blocks of [512, 64] per core
NT = NBLK // 4             # 32 tiles of 4 blocks
EPS = float(np.finfo(np.float32).eps)

# buffer depths (slots)
DXI = 6    # xin tiles in flight
DQ = 3     # sq slots
DS = 3     # ss / rms / inv slots
DXS = 3    # xs (prescaled bf16)
DP1 = 2    # psum transpose slots ([128,1024] f32, 2 banks each)
DXT = 3    # xTs (sbuf bf16)
DP2 = 2    # psum matmul banks (2 banks each)
DO = 8     # outsb slots (two 4-tile out-DMA batches)


def _build_nc(nt=NT):
    from contextlib import ExitStack

    import concourse.bass as bass
    import concourse.mybir as mybir

    f32 = mybir.dt.float32
    bf16 = mybir.dt.bfloat16
    SQRT = mybir.ActivationFunctionType.Sqrt

    nc = bass.Bass()
    nblk = 4 * nt
    x_d = nc.declare_dram_parameter("x", [nblk, N, DE], f32, isOutput=False)
    wg_d = nc.declare_dram_parameter("wg", [128, 128], bf16, isOutput=False)
    idb_d = nc.declare_dram_parameter("idb", [128, 128], bf16, isOutput=False)
    i4_d = nc.declare_dram_parameter("i4", [4, 128], bf16, isOutput=False)
    i32_d = nc.declare_dram_parameter("i32", [32, 128], bf16, isOutput=False)
    rpm_d = nc.declare_dram_parameter("rpm", [4, 64 * nt], bf16, isOutput=False)
    cpm0_d = nc.declare_dram_parameter("cpm0", [32, 1024], bf16, isOutput=False)
    cpm1_d = nc.declare_dram_parameter("cpm1", [32, 1024], bf16, isOutput=False)
    out_d = nc.declare_dram_parameter("out", [nblk, N, DE], f32, isOutput=True)

    st = ExitStack()
    with st:
        sb = lambda name, shape, dt: st.enter_context(nc.sbuf_tensor(name, shape, dt))
        wg = sb("wg_sb", [128, 128], bf16)
        idb = sb("idb_sb", [128, 128], bf16)
        i4 = sb("i4_sb", [4, 128], bf16)
        i32 = sb("i32_sb", [32, 128], bf16)
        rpm = sb("rpm_sb", [4, 64 * nt], bf16)
        cpm0 = sb("cpm0_sb", [32, 1024], bf16)
        cpm1 = sb("cpm1_sb", [32, 1024], bf16)
        epsb = sb("epsb", [128, 1], f32)
        xin = [sb(f"xin{i}", [128, 1024], f32) for i in range(DXI)]
        sq = [sb(f"sq{i}", [128, 1024], bf16) for i in range(DQ)]
        ss = [sb(f"ss{i}", [128, 16], f32) for i in range(DS)]
        rms = [sb(f"rms{i}", [128, 16], f32) for i in range(DS)]
        inv = [sb(f"inv{i}", [128, 16], f32) for i in range(DS)]
        xs = [sb(f"xs{i}", [128, 1024], bf16) for i in range(DXS)]
        xTs = [sb(f"xTs{i}", [128, 1024], bf16) for i in range(DXT)]
        outsb = sb("outsb", [128, DO * 1024], f32)
        ps1 = [
            st.enter_context(nc.psum_tensor(f"ps1{i}", [128, 1024], f32))
            for i in range(DP1)
        ]
        ps2 = [
            st.enter_context(nc.psum_tensor(f"ps2{i}", [128, 1024], f32))
            for i in range(DP2)
        ]

        sem = lambda name: st.enter_context(nc.semaphore(name))
        s_c = sem("s_c")
        s_in = [sem(f"s_in{i}") for i in range(DXI)]
        s_od = [sem(f"s_od{i}") for i in range(2)]
        s_act = sem("s_act")
        s_dve = sem("s_dve")
        s_pl = sem("s_pl")
        s_pe = sem("s_pe")

        # --- precomputed semaphore values: done[(op, t)] = sem value once
        # op(t) has retired.
        done = {}
        for t in range(nt):
            done[("in", t)] = 16 * (t // DXI + 1)      # on s_in[t % DXI]
        ng = nt // 4
        for g in range(ng):
            done[("outd", g)] = 16 * (g // 2 + 1)      # on s_od[g % 2]

        # ACT iter u: final(u-9), square(u-1), sqrt(u-3)  (final FIRST so
        # PE's ps2-slot wait resolves a full iteration early)
        c = 0
        for u in range(nt + 11):
            if 0 <= u - 9 < nt:
                c += 1
                done[("final", u - 9)] = c
            if 0 <= u - 1 < nt:
                c += 1
                done[("square", u - 1)] = c
            if 0 <= u - 3 < nt:
                c += 1
                done[("sqrt", u - 3)] = c
        # DVE iter u: reduce(u-2), recip(u-4), copy1(u-7)
        c = 0
        for u in range(nt + 8):
            if 0 <= u - 2 < nt:
                c += 1
                done[("reduce", u - 2)] = c
            if 0 <= u - 4 < nt:
                c += 1
                done[("recip", u - 4)] = c
            if 0 <= u - 7 < nt:
                c += 1
                done[("copy1", u - 7)] = c
        # Pool iter u: prescale(u-5)
        c = 0
        for u in range(nt + 6):
            if 0 <= u - 5 < nt:
                c += 1
                done[("prescale", u - 5)] = c
        # PE iter u: fwdT(u-6) x8 normal-mode identity matmuls, then
        # row/col + chunk-mms(u-8) x12. ONE s_pe inc per iteration on the
        # very last matmul -- mid-stream sem updates stall the next matmul
        # until the updating op retires, so expose none.
        c = 0
        for u in range(nt + 9):
            k = 0
            if 0 <= u - 6 < nt:
                k += 8
            if 0 <= u - 8 < nt:
                k += 12
            c += k
            if 0 <= u - 6 < nt:
                done[("fwdT", u - 6)] = c
            if 0 <= u - 8 < nt:
                done[("mms", u - 8)] = c
            done[("peiter", u)] = k

        CONST_TARGET = 7 * 16

        def in_src(t):
            return x_d[4 * t : 4 * t + 4].rearrange(
                "a (p r) e -> (a p) (r e)", p=32, r=16
            )

        def out_dst(g):
            return out_d[16 * g : 16 * g + 16].rearrange(
                "(T a) (p r) e -> (a p) T (r e)", T=4, a=4, p=32, r=16
            )

        def g16(x):
            return x.rearrange("p (g e) -> p g e", e=DE)

        with nc.Block() as block:

            @block.sync
            def _(sync):
                for cdst, csrc in (
                    (wg, wg_d),
                    (idb, idb_d),
                    (i4, i4_d),
                    (i32, i32_d),
                    (rpm, rpm_d),
                    (cpm0, cpm0_d),
                    (cpm1, cpm1_d),
                ):
                    sync.dma_start(out=cdst[:], in_=csrc[:]).then_inc(s_c, 16)
                for t in range(nt):
                    if t >= DXI:
                        sync.wait_ge(s_pl, done[("prescale", t - DXI)])
                    sync.dma_start(out=xin[t % DXI][:], in_=in_src(t)).then_inc(
                        s_in[t % DXI], 16
                    )

            @block.scalar
            def _(scalar):
                scalar.wait_ge(s_c, CONST_TARGET)
                for u in range(nt + 11):
                    if 0 <= u - 9 < nt:
                        t = u - 9
                        scalar.wait_ge(s_pe, done[("mms", t)])
                        if t >= DO:
                            gp = (t - DO) // 4
                            scalar.wait_ge(s_od[gp % 2], done[("outd", gp)])
                        nc.scalar.copy(
                            outsb[:, 1024 * (t % DO) : 1024 * (t % DO) + 1024],
                            ps2[t % DP2][:],
                        ).then_inc(s_act, 1)
                    if 0 <= u - 1 < nt:
                        t = u - 1
                        scalar.wait_ge(s_in[t % DXI], done[("in", t)])
                        if t >= DQ:
                            scalar.wait_ge(s_dve, done[("reduce", t - DQ)])
                        nc.scalar.square(sq[t % DQ][:], xin[t % DXI][:]).then_inc(
                            s_act, 1
                        )
                    if 0 <= u - 3 < nt:
                        t = u - 3
                        scalar.wait_ge(s_dve, done[("reduce", t)])
                        if t >= DS:
                            scalar.wait_ge(s_dve, done[("recip", t - DS)])
                        nc.scalar.activation(
                            rms[t % DS][:], ss[t % DS][:], SQRT,
                            bias=epsb[:], scale=1.0 / DE,
                        ).then_inc(s_act, 1)

            @block.vector
            def _(vector):
                vector.wait_ge(s_c, CONST_TARGET)
                nc.vector.memset(epsb[:], EPS)
                for u in range(nt + 8):
                    if 0 <= u - 2 < nt:
                        t = u - 2
                        vector.wait_ge(s_act, done[("square", t)])
                        if t >= DS:
                            vector.wait_ge(s_act, done[("sqrt", t - DS)])
                        nc.vector.tensor_reduce(
                            ss[t % DS][:],
                            g16(sq[t % DQ][:]),
                            axis=mybir.AxisListType.X,
                            op=mybir.AluOpType.add,
                        ).then_inc(s_dve, 1)
                    if 0 <= u - 4 < nt:
                        t = u - 4
                        vector.wait_ge(s_act, done[("sqrt", t)])
                        if t >= DS:
                            vector.wait_ge(s_pl, done[("prescale", t - DS)])
                        nc.vector.reciprocal(inv[t % DS][:], rms[t % DS][:]).then_inc(
                            s_dve, 1
                        )
                    if 0 <= u - 7 < nt:
                        t = u - 7
                        vector.wait_ge(s_pe, done[("fwdT", t)])
                        if t >= DXT:
                            vector.wait_ge(s_pe, done[("mms", t - DXT)])
                        nc.vector.tensor_copy(
                            xTs[t % DXT][:], ps1[t % DP1][:]
                        ).then_inc(s_dve, 1)

            @block.gpsimd
            def _(pool):
                pool.wait_ge(s_c, CONST_TARGET)
                for u in range(nt + 15):
                    if 0 <= u - 5 < nt:
                        t = u - 5
                        pool.wait_ge(s_dve, done[("recip", t)])
                        if t >= DXS:
                            pool.wait_ge(s_pe, done[("fwdT", t - DXS)])
                        nc.gpsimd.tensor_mul(
                            g16(xs[t % DXS][:]),
                            g16(xin[t % DXI][:]),
                            inv[t % DS][:].unsqueeze(-1).broadcast_to([128, 16, DE]),
                        ).then_inc(s_pl, 1)
                    if u >= 13 and (u - 13) % 4 == 0 and (u - 13) // 4 < ng:
                        g = (u - 13) // 4
                        pool.wait_ge(s_act, done[("final", 4 * g + 3)])
                        half = 4096 * ((4 * g) % DO // 4)
                        pool.dma_start(
                            out=out_dst(g), in_=outsb[:, half : half + 4096]
                        ).then_inc(s_od[g % 2], 16)

            @block.tensor
            def _(tensor):
                tensor.wait_ge(s_c, CONST_TARGET)
                for u in range(nt + 9):
                    tT = u - 6
                    tM = u - 8
                    doT = 0 <= tT < nt
                    doM = 0 <= tM < nt
                    inc = done[("peiter", u)]
                    if doT:
                        tensor.wait_ge(s_pl, done[("prescale", tT)])
                        if tT >= DP1:
                            tensor.wait_ge(s_dve, done[("copy1", tT - DP1)])
                        for c in range(8):
                            mi = nc.tensor.matmul(
                                ps1[tT % DP1][:, 128 * c : 128 * c + 128],
                                xs[tT % DXS][:, 128 * c : 128 * c + 128],
                                idb[:],
                                start=True, stop=True, skip_group_check=True,
                            )
                            if c == 7 and not doM:
                                mi.then_inc(s_pe, inc)
                    if doM:
                        if tM >= DP2:
                            tensor.wait_ge(s_act, done[("final", tM - DP2)])
                        pt = ps2[tM % DP2]
                        rp = (
                            rpm[:, 64 * tM : 64 * tM + 64]
                            .unsqueeze(1)
                            .broadcast_to([4, 8, DE])
                        )
                        cpm = cpm0 if tM < nt // 2 else cpm1
                        for h in range(2):
                            nc.tensor.matmul(
                                pt[:, 512 * h : 512 * h + 512], i4[:], rp,
                                start=True, stop=False, skip_group_check=True,
                            )
                        for h in range(2):
                            nc.tensor.matmul(
                                pt[:, 512 * h : 512 * h + 512],
                                i32[:],
                                cpm[:, 512 * h : 512 * h + 512],
                                start=False, stop=False, skip_group_check=True,
                            )
                        tensor.wait_ge(s_dve, done[("copy1", tM)])
                        for c in range(8):
                            mi = nc.tensor.matmul(
                                pt[:, 128 * c : 128 * c + 128],
                                xTs[tM % DXT][:, 128 * c : 128 * c + 128],
                                wg[:],
                                start=False, stop=True, skip_group_check=True,
                            )
                            if c == 7:
                                mi.then_inc(s_pe, inc)

    return nc


_NC_CACHE = {}


def _get_nc():
    if "nc" not in _NC_CACHE:
        _NC_CACHE["nc"] = _build_nc()
    return _NC_CACHE["nc"]


def _make_in_maps(edges, nodes, g_node, g_edge, W, b):
    import ml_dtypes

    bf = ml_dtypes.bfloat16
    edges = np.ascontiguousarray(edges, dtype=np.float32)
    nodes = np.ascontiguousarray(nodes, dtype=np.float32)

    # ---- host: tiny node path (B*N*dn = 131K elems)
    ms = np.mean(np.square(nodes), axis=-1, keepdims=True)
    nodes_n = nodes / np.sqrt(ms + EPS) * g_node  # [B, N, 128]
    Wr, Wc, We = W[:DN], W[DN : 2 * DN], W[2 * DN :]
    row_proj = (nodes_n @ Wr).astype(np.float32)  # [B, N, 64]
    col_proj = (nodes_n @ Wc).astype(np.float32)  # [B, N, 64]
    Wg = (g_edge[:, None] * We).astype(np.float32)  # fold g_edge into We

    # block-diagonal Wg (even-r rows top-left, odd-r rows bottom-right)
    wgblk = np.zeros((128, 128), dtype=np.float32)
    wgblk[:64, :64] = Wg
    wgblk[64:, 64:] = Wg
    wgblk = wgblk.astype(bf)
    idb = np.eye(128, dtype=np.float32).astype(bf)
    i4 = np.kron(np.eye(4, dtype=np.float32), np.ones((1, 32), np.float32)).astype(bf)
    i32 = np.tile(np.eye(32, dtype=np.float32), (1, 4)).astype(bf)

    # col_proj + bias, per b: cpm_b[p', 64r + e'] = cp_b[16 p' + r, e']
    cp = (col_proj + b).astype(np.float32)  # [B, 512, 64]
    cpm0 = np.ascontiguousarray(cp[0].reshape(32, 1024)).astype(bf)
    cpm1 = np.ascontiguousarray(cp[1].reshape(32, 1024)).astype(bf)

    in_maps = []
    for c in range(NCORES):
        xc = edges[:, c * IPC : (c + 1) * IPC]  # [B, 64, 512, 64]
        xc = np.ascontiguousarray(xc).reshape(NBLK, N, DE)
        # rpm[k, 64T + e'] = row_proj[block 4T + k][e']
        rp = row_proj[:, c * IPC : (c + 1) * IPC].reshape(NBLK, DE)  # [128, 64]
        rpm = np.ascontiguousarray(
            rp.reshape(NT, 4, DE).transpose(1, 0, 2).reshape(4, NT * DE)
        ).astype(bf)
        in_maps.append(
            {
                "x": xc,
                "wg": wgblk,
                "idb": idb,
                "i4": i4,
                "i32": i32,
                "rpm": rpm,
                "cpm0": cpm0,
                "cpm1": cpm1,
            }
        )
    return in_maps


def _run(edges, nodes, g_node, g_edge, W, b, trace=False, **spmd_kwargs):
    in_maps = _make_in_maps(edges, nodes, g_node, g_edge, W, b)

    from concourse.bass_utils import run_bass_kernel_spmd

    nc = _get_nc()
    res = run_bass_kernel_spmd(
        nc, in_maps, list(range(NCORES)), trace=trace, **spmd_kwargs
    )

    out = np.empty((B, N, N, DE), dtype=np.float32)
    for c in range(NCORES):
        oc = res.results[c]["out"].reshape(B, IPC, N, DE)
        out[:, c * IPC : (c + 1) * IPC] = oc
    return out, res


def kernel(edges, nodes, g_node, g_edge, W, b):
    return _run(edges, nodes, g_node, g_edge, W, b)[0]


if __name__ == "__main__":
    rng = np.random.default_rng(0)
    edges = rng.standard_normal((B, N, N, DE), dtype=np.float32)
    nodes = rng.standard_normal((B, N, DN), dtype=np.float32)
    g_node = np.ones(DN, np.float32)
    g_edge = np.ones(DE, np.float32)
    W = rng.standard_normal((2 * DN + DE, DE), dtype=np.float32) / 18.0
    b = (rng.standard_normal(DE) * 0.01).astype(np.float32)
    o = kernel(edges, nodes, g_node, g_edge, W, b)
    print(o.shape, o.dtype)


# revision 16
# speedup vs baseline: 1.1671x; 1.0059x over previous
"""Bass/Trainium2 kernel for nn_NodesToEdges (gnn_message_passing).

out[b,i,j,:] = rms(edges[b,i,j,:])*g_e @ We + rms(nodes[b,i,:])*g_n @ Wr
             + rms(nodes[b,j,:])*g_n @ Wc + bias

Strategy: shard over i (rows) across 8 cores. The tiny node path
(row_proj / col_proj) is precomputed on host. The edge path (134 MB in /
134 MB out, memory-bound) runs on device.

Per core: 128 blocks (b,i) of [512 j, 64 e], processed as 32 TILES of 4
blocks = [128 part = (a,p=j//16), 1024 free = (r=j%16, e)] (4 KB DMA
descriptors). Per tile:
  ACT: square -> DVE: 16-group reduce -> ACT: sqrt(mean+eps) ->
  DVE: recip -> Pool: prescale (x*inv -> bf16) ->
  PE: 8x transpose (128-col chunks, bf16) -> DVE: copy psum->sbuf ->
  PE: row/col adds via indicator-stationary matmuls + 8 chunk-matmuls
      with the TRANSPOSED DATA as stationary (output lands row-major,
      f32 psum; no back-transpose) ->
  ACT: copy psum->sbuf -> Pool: SWDGE out-DMA (4-tile batches).
In-DMA on the SP HWDGE ring; out-DMA on the Pool SWDGE ring.
"""

import numpy as np

B, N, DE, DN = 2, 512, 64, 128
NCORES = 8
IPC = N // NCORES          # 64 i-rows per core
NBLK = B * IPC             # 128 blocks of [512, 64] per core
NT = NBLK // 4             # 32 tiles of 4 blocks
EPS = float(np.finfo(np.float32).eps)

# buffer depths (slots)
DXI = 6    # xin tiles in flight
DQ = 3     # sq slots
DS = 3     # ss / rms / inv slots
DXS = 3    # xs (prescaled bf16)
DP1 = 2    # psum transpose slots ([128,1024] f32, 2 banks each)
DXT = 3    # xTs (sbuf bf16)
DP2 = 2    # psum matmul banks (2 banks each)
DO = 8     # outsb slots (two 4-tile out-DMA batches)


def _build_nc(nt=NT):
    from contextlib import ExitStack

    import concourse.bass as bass
    import concourse.mybir as mybir

    f32 = mybir.dt.float32
    bf16 = mybir.dt.bfloat16
    SQRT = mybir.ActivationFunctionType.Sqrt

    nc = bass.Bass()
    nblk = 4 * nt
    x_d = nc.declare_dram_parameter("x", [nblk, N, DE], f32, isOutput=False)
    wg_d = nc.declare_dram_parameter("wg", [128, 128], bf16, isOutput=False)
    idb_d = nc.declare_dram_parameter("idb", [128, 128], bf16, isOutput=False)
    i4_d = nc.declare_dram_parameter("i4", [4, 128], bf16, isOutput=False)
    i32_d = nc.declare_dram_parameter("i32", [32, 128], bf16, isOutput=False)
    rpm_d = nc.declare_dram_parameter("rpm", [4, 64 * nt], bf16, isOutput=False)
    cpm0_d = nc.declare_dram_parameter("cpm0", [32, 1024], bf16, isOutput=False)
    cpm1_d = nc.declare_dram_parameter("cpm1", [32, 1024], bf16, isOutput=False)
    out_d = nc.declare_dram_parameter("out", [nblk, N, DE], f32, isOutput=True)

    st = ExitStack()
    with st:
        sb = lambda name, shape, dt: st.enter_context(nc.sbuf_tensor(name, shape, dt))
        wg = sb("wg_sb", [128, 128], bf16)
        idb = sb("idb_sb", [128, 128], bf16)
        i4 = sb("i4_sb", [4, 128], bf16)
        i32 = sb("i32_sb", [32, 128], bf16)
        rpm = sb("rpm_sb", [4, 64 * nt], bf16)
        cpm0 = sb("cpm0_sb", [32, 1024], bf16)
        cpm1 = sb("cpm1_sb", [32, 1024], bf16)
        epsb = sb("epsb", [128, 1], f32)
        xin = [sb(f"xin{i}", [128, 1024], f32) for i in range(DXI)]
        sq = [sb(f"sq{i}", [128, 1024], bf16) for i in range(DQ)]
        ss = [sb(f"ss{i}", [128, 16], f32) for i in range(DS)]
        rms = [sb(f"rms{i}", [128, 16], f32) for i in range(DS)]
        inv = [sb(f"inv{i}", [128, 16], f32) for i in range(DS)]
        xs = [sb(f"xs{i}", [128, 1024], bf16) for i in range(DXS)]
        xTs = [sb(f"xTs{i}", [128, 1024], bf16) for i in range(DXT)]
        outsb = sb("outsb", [128, DO * 1024], f32)
        ps1 = [
            st.enter_context(nc.psum_tensor(f"ps1{i}", [128, 1024], f32))
            for i in range(DP1)
        ]
        ps2 = [
            st.enter_context(nc.psum_tensor(f"ps2{i}", [128, 1024], f32))
            for i in range(DP2)
        ]

        sem = lambda name: st.enter_context(nc.semaphore(name))
        s_c = sem("s_c")
        s_in = [sem(f"s_in{i}") for i in range(DXI)]
        s_od = [sem(f"s_od{i}") for i in range(2)]
        s_act = sem("s_act")
        s_dve = sem("s_dve")
        s_pl = sem("s_pl")
        s_pe = sem("s_pe")

        # --- precomputed semaphore values: done[(op, t)] = sem value once
        # op(t) has retired.
        done = {}
        for t in range(nt):
            done[("in", t)] = 16 * (t // DXI + 1)      # on s_in[t % DXI]
        ng = nt // 4
        for g in range(ng):
            done[("outd", g)] = 16 * (g // 2 + 1)      # on s_od[g % 2]

        # ACT iter u: final(u-9), square(u-1), sqrt(u-3)  (final FIRST so
        # PE's ps2-slot wait resolves a full iteration early)
        c = 0
        for u in range(nt + 11):
            if 0 <= u - 9 < nt:
                c += 1
                done[("final", u - 9)] = c
            if 0 <= u - 1 < nt:
                c += 1
                done[("square", u - 1)] = c
            if 0 <= u - 3 < nt:
                c += 1
                done[("sqrt", u - 3)] = c
        # DVE iter u: reduce(u-2), recip(u-4), copy1(u-7)
        c = 0
        for u in range(nt + 8):
            if 0 <= u - 2 < nt:
                c += 1
                done[("reduce", u - 2)] = c
            if 0 <= u - 4 < nt:
                c += 1
                done[("recip", u - 4)] = c
            if 0 <= u - 7 < nt:
                c += 1
                done[("copy1", u - 7)] = c
        # Pool iter u: prescale(u-5)
        c = 0
        for u in range(nt + 6):
            if 0 <= u - 5 < nt:
                c += 1
                done[("prescale", u - 5)] = c
        # PE iter u: fwdT(u-6) x8 normal-mode identity matmuls, then
        # row/col + chunk-mms(u-8) x12. ONE s_pe inc per iteration on the
        # very last matmul -- mid-stream sem updates stall the next matmul
        # until the updating op retires, so expose none.
        c = 0
        for u in range(nt + 9):
            k = 0
            if 0 <= u - 6 < nt:
                k += 8
            if 0 <= u - 8 < nt:
                k += 12
            c += k
            if 0 <= u - 6 < nt:
                done[("fwdT", u - 6)] = c
            if 0 <= u - 8 < nt:
                done[("mms", u - 8)] = c
            done[("peiter", u)] = k

        CONST_TARGET = 7 * 16

        def in_src(t):
            return x_d[4 * t : 4 * t + 4].rearrange(
                "a (p r) e -> (a p) (r e)", p=32, r=16
            )

        def out_dst(g):
            return out_d[16 * g : 16 * g + 16].rearrange(
                "(T a) (p r) e -> (a p) T (r e)", T=4, a=4, p=32, r=16
            )

        def g16(x):
            return x.rearrange("p (g e) -> p g e", e=DE)

        with nc.Block() as block:

            @block.sync
            def _(sync):
                for cdst, csrc in (
                    (wg, wg_d),
                    (idb, idb_d),
                    (i4, i4_d),
                    (i32, i32_d),
                    (rpm, rpm_d),
                    (cpm0, cpm0_d),
                    (cpm1, cpm1_d),
                ):
                    sync.dma_start(out=cdst[:], in_=csrc[:]).then_inc(s_c, 16)
                for t in range(nt):
                    if t >= DXI:
                        sync.wait_ge(s_pl, done[("prescale", t - DXI)])
                    sync.dma_start(out=xin[t % DXI][:], in_=in_src(t)).then_inc(
                        s_in[t % DXI], 16
                    )

            @block.scalar
            def _(scalar):
                scalar.wait_ge(s_c, CONST_TARGET)
                for u in range(nt + 11):
                    if 0 <= u - 9 < nt:
                        t = u - 9
                        scalar.wait_ge(s_pe, done[("mms", t)])
                        if t >= DO:
                            gp = (t - DO) // 4
                            scalar.wait_ge(s_od[gp % 2], done[("outd", gp)])
                        nc.scalar.copy(
                            outsb[:, 1024 * (t % DO) : 1024 * (t % DO) + 1024],
                            ps2[t % DP2][:],
                        ).then_inc(s_act, 1)
                    if 0 <= u - 1 < nt:
                        t = u - 1
                        scalar.wait_ge(s_in[t % DXI], done[("in", t)])
                        if t >= DQ:
                            scalar.wait_ge(s_dve, done[("reduce", t - DQ)])
                        nc.scalar.square(
                            sq[t % DQ][:], xin[t % DXI][:].bitcast(bf16)[:, 1::2]
                        ).then_inc(s_act, 1)
                    if 0 <= u - 3 < nt:
                        t = u - 3
                        scalar.wait_ge(s_dve, done[("reduce", t)])
                        if t >= DS:
                            scalar.wait_ge(s_dve, done[("recip", t - DS)])
                        nc.scalar.activation(
                            rms[t % DS][:], ss[t % DS][:], SQRT,
                            bias=epsb[:], scale=1.0 / DE,
                        ).then_inc(s_act, 1)

            @block.vector
            def _(vector):
                vector.wait_ge(s_c, CONST_TARGET)
                nc.vector.memset(epsb[:], EPS)
                for u in range(nt + 8):
                    if 0 <= u - 2 < nt:
                        t = u - 2
                        vector.wait_ge(s_act, done[("square", t)])
                        if t >= DS:
                            vector.wait_ge(s_act, done[("sqrt", t - DS)])
                        nc.vector.tensor_reduce(
                            ss[t % DS][:],
                            g16(sq[t % DQ][:]),
                            axis=mybir.AxisListType.X,
                            op=mybir.AluOpType.add,
                        ).then_inc(s_dve, 1)
                    if 0 <= u - 4 < nt:
                        t = u - 4
                        vector.wait_ge(s_act, done[("sqrt", t)])
                        if t >= DS:
                            vector.wait_ge(s_pl, done[("prescale", t - DS)])
                        nc.vector.reciprocal(inv[t % DS][:], rms[t % DS][:]).then_inc(
                            s_dve, 1
                        )
                    if 0 <= u - 7 < nt:
                        t = u - 7
                        vector.wait_ge(s_pe, done[("fwdT", t)])
                        if t >= DXT:
                            vector.wait_ge(s_pe, done[("mms", t - DXT)])
                        nc.vector.tensor_copy(
                            xTs[t % DXT][:], ps1[t % DP1][:]
                        ).then_inc(s_dve, 1)

            @block.gpsimd
            def _(pool):
                pool.wait_ge(s_c, CONST_TARGET)
                for u in range(nt + 15):
                    if 0 <= u - 5 < nt:
                        t = u - 5
                        pool.wait_ge(s_dve, done[("recip", t)])
                        if t >= DXS:
                            pool.wait_ge(s_pe, done[("fwdT", t - DXS)])
                        nc.gpsimd.tensor_mul(
                            g16(xs[t % DXS][:]),
                            g16(xin[t % DXI][:]),
                            inv[t % DS][:].unsqueeze(-1).broadcast_to([128, 16, DE]),
                        ).then_inc(s_pl, 1)
                    if u >= 13 and (u - 13) % 4 == 0 and (u - 13) // 4 < ng:
                        g = (u - 13) // 4
                        pool.wait_ge(s_act, done[("final", 4 * g + 3)])
                        half = 4096 * ((4 * g) % DO // 4)
                        pool.dma_start(
                            out=out_dst(g), in_=outsb[:, half : half + 4096]
                        ).then_inc(s_od[g % 2], 16)

            @block.tensor
            def _(tensor):
                tensor.wait_ge(s_c, CONST_TARGET)
                for u in range(nt + 9):
                    tT = u - 6
                    tM = u - 8
                    doT = 0 <= tT < nt
                    doM = 0 <= tM < nt
                    inc = done[("peiter", u)]
                    if doT:
                        tensor.wait_ge(s_pl, done[("prescale", tT)])
                        if tT >= DP1:
                            tensor.wait_ge(s_dve, done[("copy1", tT - DP1)])
                        for c in range(8):
                            mi = nc.tensor.matmul(
                                ps1[tT % DP1][:, 128 * c : 128 * c + 128],
                                xs[tT % DXS][:, 128 * c : 128 * c + 128],
                                idb[:],
                                start=True, stop=True, skip_group_check=True,
                            )
                            if c == 7 and not doM:
                                mi.then_inc(s_pe, inc)
                    if doM:
                        if tM >= DP2:
                            tensor.wait_ge(s_act, done[("final", tM - DP2)])
                        pt = ps2[tM % DP2]
                        rp = (
                            rpm[:, 64 * tM : 64 * tM + 64]
                            .unsqueeze(1)
                            .broadcast_to([4, 8, DE])
                        )
                        cpm = cpm0 if tM < nt // 2 else cpm1
                        for h in range(2):
                            nc.tensor.matmul(
                                pt[:, 512 * h : 512 * h + 512], i4[:], rp,
                                start=True, stop=False, skip_group_check=True,
                            )
                        for h in range(2):
                            nc.tensor.matmul(
                                pt[:, 512 * h : 512 * h + 512],
                                i32[:],
                                cpm[:, 512 * h : 512 * h + 512],
                                start=False, stop=False, skip_group_check=True,
                            )
                        tensor.wait_ge(s_dve, done[("copy1", tM)])
                        for c in range(8):
                            mi = nc.tensor.matmul(
                                pt[:, 128 * c : 128 * c + 128],
                                xTs[tM % DXT][:, 128 * c : 128 * c + 128],
                                wg[:],
                                start=False, stop=True, skip_group_check=True,
                            )
                            if c == 7:
                                mi.then_inc(s_pe, inc)

    return nc


_NC_CACHE = {}


def _get_nc():
    if "nc" not in _NC_CACHE:
        _NC_CACHE["nc"] = _build_nc()
    return _NC_CACHE["nc"]


def _make_in_maps(edges, nodes, g_node, g_edge, W, b):
    import ml_dtypes

    bf = ml_dtypes.bfloat16
    edges = np.ascontiguousarray(edges, dtype=np.float32)
    nodes = np.ascontiguousarray(nodes, dtype=np.float32)

    # ---- host: tiny node path (B*N*dn = 131K elems)
    ms = np.mean(np.square(nodes), axis=-1, keepdims=True)
    nodes_n = nodes / np.sqrt(ms + EPS) * g_node  # [B, N, 128]
    Wr, Wc, We = W[:DN], W[DN : 2 * DN], W[2 * DN :]
    row_proj = (nodes_n @ Wr).astype(np.float32)  # [B, N, 64]
    col_proj = (nodes_n @ Wc).astype(np.float32)  # [B, N, 64]
    Wg = (g_edge[:, None] * We).astype(np.float32)  # fold g_edge into We

    # block-diagonal Wg (even-r rows top-left, odd-r rows bottom-right)
    wgblk = np.zeros((128, 128), dtype=np.float32)
    wgblk[:64, :64] = Wg
    wgblk[64:, 64:] = Wg
    wgblk = wgblk.astype(bf)
    idb = np.eye(128, dtype=np.float32).astype(bf)
    i4 = np.kron(np.eye(4, dtype=np.float32), np.ones((1, 32), np.float32)).astype(bf)
    i32 = np.tile(np.eye(32, dtype=np.float32), (1, 4)).astype(bf)

    # col_proj + bias, per b: cpm_b[p', 64r + e'] = cp_b[16 p' + r, e']
    cp = (col_proj + b).astype(np.float32)  # [B, 512, 64]
    cpm0 = np.ascontiguousarray(cp[0].reshape(32, 1024)).astype(bf)
    cpm1 = np.ascontiguousarray(cp[1].reshape(32, 1024)).astype(bf)

    in_maps = []
    for c in range(NCORES):
        xc = edges[:, c * IPC : (c + 1) * IPC]  # [B, 64, 512, 64]
        xc = np.ascontiguousarray(xc).reshape(NBLK, N, DE)
        # rpm[k, 64T + e'] = row_proj[block 4T + k][e']
        rp = row_proj[:, c * IPC : (c + 1) * IPC].reshape(NBLK, DE)  # [128, 64]
        rpm = np.ascontiguousarray(
            rp.reshape(NT, 4, DE).transpose(1, 0, 2).reshape(4, NT * DE)
        ).astype(bf)
        in_maps.append(
            {
                "x": xc,
                "wg": wgblk,
                "idb": idb,
                "i4": i4,
                "i32": i32,
                "rpm": rpm,
                "cpm0": cpm0,
                "cpm1": cpm1,
            }
        )
    return in_maps


def _run(edges, nodes, g_node, g_edge, W, b, trace=False, **spmd_kwargs):
    in_maps = _make_in_maps(edges, nodes, g_node, g_edge, W, b)

    from concourse.bass_utils import run_bass_kernel_spmd

    nc = _get_nc()
    res = run_bass_kernel_spmd(
        nc, in_maps, list(range(NCORES)), trace=trace, **spmd_kwargs
    )

    out = np.empty((B, N, N, DE), dtype=np.float32)
    for c in range(NCORES):
        oc = res.results[c]["out"].reshape(B, IPC, N, DE)
        out[:, c * IPC : (c + 1) * IPC] = oc
    return out, res


def kernel(edges, nodes, g_node, g_edge, W, b):
    return _run(edges, nodes, g_node, g_edge, W, b)[0]


if __name__ == "__main__":
    rng = np.random.default_rng(0)
    edges = rng.standard_normal((B, N, N, DE), dtype=np.float32)
    nodes = rng.standard_normal((B, N, DN), dtype=np.float32)
    g_node = np.ones(DN, np.float32)
    g_edge = np.ones(DE, np.float32)
    W = rng.standard_normal((2 * DN + DE, DE), dtype=np.float32) / 18.0
    b = (rng.standard_normal(DE) * 0.01).astype(np.float32)
    o = kernel(edges, nodes, g_node, g_edge, W, b)
    print(o.shape, o.dtype)


# revision 17
# speedup vs baseline: 1.1916x; 1.0210x over previous
"""Bass/Trainium2 kernel for nn_NodesToEdges (gnn_message_passing).

out[b,i,j,:] = rms(edges[b,i,j,:])*g_e @ We + rms(nodes[b,i,:])*g_n @ Wr
             + rms(nodes[b,j,:])*g_n @ Wc + bias

Strategy: shard over i (rows) across 8 cores. The tiny node path
(row_proj / col_proj) is precomputed on host. The edge path (134 MB in /
134 MB out, memory-bound) runs on device.

Per core: 128 blocks (b,i) of [512 j, 64 e], processed as 32 TILES of 4
blocks = [128 part = (a,p=j//16), 1024 free = (r=j%16, e)] (4 KB DMA
descriptors). Per tile:
  ACT: square -> DVE: 16-group reduce -> ACT: sqrt(mean+eps) ->
  DVE: recip -> Pool: prescale (x*inv -> bf16) ->
  PE: 8x transpose (128-col chunks, bf16) -> DVE: copy psum->sbuf ->
  PE: row/col adds via indicator-stationary matmuls + 8 chunk-matmuls
      with the TRANSPOSED DATA as stationary (output lands row-major,
      f32 psum; no back-transpose) ->
  ACT: copy psum->sbuf -> Pool: SWDGE out-DMA (4-tile batches).
In-DMA on the SP HWDGE ring; out-DMA on the Pool SWDGE ring.
"""

import numpy as np

B, N, DE, DN = 2, 512, 64, 128
NCORES = 8
IPC = N // NCORES          # 64 i-rows per core
NBLK = B * IPC             # 128 blocks of [512, 64] per core
NT = NBLK // 4             # 32 tiles of 4 blocks
EPS = float(np.finfo(np.float32).eps)

# buffer depths (slots)
DXI = 6    # xin tiles in flight
DQ = 3     # sq slots
DS = 3     # ss / rms / inv slots
DXS = 3    # xs (prescaled bf16)
DP1 = 2    # psum transpose slots ([128,1024] f32, 2 banks each)
DXT = 3    # xTs (sbuf bf16)
DP2 = 2    # psum matmul banks (2 banks each)
DO = 8     # outsb slots (two 4-tile out-DMA batches)


def _build_nc(nt=NT):
    from contextlib import ExitStack

    import concourse.bass as bass
    import concourse.mybir as mybir

    f32 = mybir.dt.float32
    bf16 = mybir.dt.bfloat16
    SQRT = mybir.ActivationFunctionType.Sqrt

    nc = bass.Bass()
    nblk = 4 * nt
    x_d = nc.declare_dram_parameter("x", [nblk, N, DE], f32, isOutput=False)
    wg_d = nc.declare_dram_parameter("wg", [128, 128], bf16, isOutput=False)
    idb_d = nc.declare_dram_parameter("idb", [128, 128], bf16, isOutput=False)
    i4_d = nc.declare_dram_parameter("i4", [4, 128], bf16, isOutput=False)
    i32_d = nc.declare_dram_parameter("i32", [32, 128], bf16, isOutput=False)
    rpm_d = nc.declare_dram_parameter("rpm", [4, 64 * nt], bf16, isOutput=False)
    cpm0_d = nc.declare_dram_parameter("cpm0", [32, 1024], bf16, isOutput=False)
    cpm1_d = nc.declare_dram_parameter("cpm1", [32, 1024], bf16, isOutput=False)
    out_d = nc.declare_dram_parameter("out", [nblk, N, DE], f32, isOutput=True)

    st = ExitStack()
    with st:
        sb = lambda name, shape, dt: st.enter_context(nc.sbuf_tensor(name, shape, dt))
        wg = sb("wg_sb", [128, 128], bf16)
        idb = sb("idb_sb", [128, 128], bf16)
        i4 = sb("i4_sb", [4, 128], bf16)
        i32 = sb("i32_sb", [32, 128], bf16)
        rpm = sb("rpm_sb", [4, 64 * nt], bf16)
        cpm0 = sb("cpm0_sb", [32, 1024], bf16)
        cpm1 = sb("cpm1_sb", [32, 1024], bf16)
        epsb = sb("epsb", [128, 1], f32)
        xin = [sb(f"xin{i}", [128, 1024], f32) for i in range(DXI)]
        sq = [sb(f"sq{i}", [128, 1024], bf16) for i in range(DQ)]
        ss = [sb(f"ss{i}", [128, 16], f32) for i in range(DS)]
        rms = [sb(f"rms{i}", [128, 16], f32) for i in range(DS)]
        inv = [sb(f"inv{i}", [128, 16], f32) for i in range(DS)]
        xs = [sb(f"xs{i}", [128, 1024], bf16) for i in range(DXS)]
        xTs = [sb(f"xTs{i}", [128, 1024], bf16) for i in range(DXT)]
        outsb = sb("outsb", [128, DO * 1024], f32)
        ps1 = [
            st.enter_context(nc.psum_tensor(f"ps1{i}", [128, 1024], f32))
            for i in range(DP1)
        ]
        ps2 = [
            st.enter_context(nc.psum_tensor(f"ps2{i}", [128, 1024], f32))
            for i in range(DP2)
        ]

        sem = lambda name: st.enter_context(nc.semaphore(name))
        s_c = sem("s_c")
        s_in = [sem(f"s_in{i}") for i in range(DXI)]
        s_od = [sem(f"s_od{i}") for i in range(2)]
        s_act = sem("s_act")
        s_dve = sem("s_dve")
        s_pl = sem("s_pl")
        s_pe = sem("s_pe")

        # --- precomputed semaphore values: done[(op, t)] = sem value once
        # op(t) has retired.
        done = {}
        for t in range(nt):
            done[("in", t)] = 16 * (t // DXI + 1)      # on s_in[t % DXI]
        ng = nt // 4
        for g in range(ng):
            done[("outd", g)] = 16 * (g // 2 + 1)      # on s_od[g % 2]

        # ACT iter u: square(u-1), sqrt(u-3), final(u-9)
        c = 0
        for u in range(nt + 11):
            if 0 <= u - 1 < nt:
                c += 1
                done[("square", u - 1)] = c
            if 0 <= u - 3 < nt:
                c += 1
                done[("sqrt", u - 3)] = c
            if 0 <= u - 9 < nt:
                c += 1
                done[("final", u - 9)] = c
        # DVE iter u: reduce(u-2), recip(u-4), copy1(u-7)
        c = 0
        for u in range(nt + 8):
            if 0 <= u - 2 < nt:
                c += 1
                done[("reduce", u - 2)] = c
            if 0 <= u - 4 < nt:
                c += 1
                done[("recip", u - 4)] = c
            if 0 <= u - 7 < nt:
                c += 1
                done[("copy1", u - 7)] = c
        # Pool iter u: prescale(u-5)
        c = 0
        for u in range(nt + 6):
            if 0 <= u - 5 < nt:
                c += 1
                done[("prescale", u - 5)] = c
        # PE iter u: fwdT(u-6) x8 normal-mode identity matmuls, then
        # row/col + chunk-mms(u-8) x12. ONE s_pe inc per iteration on the
        # very last matmul -- mid-stream sem updates stall the next matmul
        # until the updating op retires, so expose none.
        c = 0
        for u in range(nt + 9):
            k = 0
            if 0 <= u - 6 < nt:
                k += 8
            if 0 <= u - 8 < nt:
                k += 12
            c += k
            if 0 <= u - 6 < nt:
                done[("fwdT", u - 6)] = c
            if 0 <= u - 8 < nt:
                done[("mms", u - 8)] = c
            done[("peiter", u)] = k

        CONST_TARGET = 7 * 16

        def in_src(t):
            return x_d[4 * t : 4 * t + 4].rearrange(
                "a (p r) e -> (a p) (r e)", p=32, r=16
            )

        def out_dst(g):
            return out_d[16 * g : 16 * g + 16].rearrange(
                "(T a) (p r) e -> (a p) T (r e)", T=4, a=4, p=32, r=16
            )

        def g16(x):
            return x.rearrange("p (g e) -> p g e", e=DE)

        with nc.Block() as block:

            @block.sync
            def _(sync):
                for cdst, csrc in (
                    (wg, wg_d),
                    (idb, idb_d),
                    (i4, i4_d),
                    (i32, i32_d),
                    (rpm, rpm_d),
                    (cpm0, cpm0_d),
                    (cpm1, cpm1_d),
                ):
                    sync.dma_start(out=cdst[:], in_=csrc[:]).then_inc(s_c, 16)
                for t in range(nt):
                    if t >= DXI:
                        sync.wait_ge(s_pl, done[("prescale", t - DXI)])
                    sync.dma_start(out=xin[t % DXI][:], in_=in_src(t)).then_inc(
                        s_in[t % DXI], 16
                    )

            @block.scalar
            def _(scalar):
                scalar.wait_ge(s_c, CONST_TARGET)
                for u in range(nt + 11):
                    if 0 <= u - 1 < nt:
                        t = u - 1
                        scalar.wait_ge(s_in[t % DXI], done[("in", t)])
                        if t >= DQ:
                            scalar.wait_ge(s_dve, done[("reduce", t - DQ)])
                        nc.scalar.square(
                            sq[t % DQ][:], xin[t % DXI][:].bitcast(bf16)[:, 1::2]
                        ).then_inc(s_act, 1)
                    if 0 <= u - 3 < nt:
                        t = u - 3
                        scalar.wait_ge(s_dve, done[("reduce", t)])
                        if t >= DS:
                            scalar.wait_ge(s_dve, done[("recip", t - DS)])
                        nc.scalar.activation(
                            rms[t % DS][:], ss[t % DS][:], SQRT,
                            bias=epsb[:], scale=1.0 / DE,
                        ).then_inc(s_act, 1)
                    if 0 <= u - 9 < nt:
                        t = u - 9
                        scalar.wait_ge(s_pe, done[("mms", t)])
                        if t >= DO:
                            gp = (t - DO) // 4
                            scalar.wait_ge(s_od[gp % 2], done[("outd", gp)])
                        nc.scalar.copy(
                            outsb[:, 1024 * (t % DO) : 1024 * (t % DO) + 1024],
                            ps2[t % DP2][:],
                        ).then_inc(s_act, 1)

            @block.vector
            def _(vector):
                vector.wait_ge(s_c, CONST_TARGET)
                nc.vector.memset(epsb[:], EPS)
                for u in range(nt + 8):
                    if 0 <= u - 2 < nt:
                        t = u - 2
                        vector.wait_ge(s_act, done[("square", t)])
                        if t >= DS:
                            vector.wait_ge(s_act, done[("sqrt", t - DS)])
                        nc.vector.tensor_reduce(
                            ss[t % DS][:],
                            g16(sq[t % DQ][:]),
                            axis=mybir.AxisListType.X,
                            op=mybir.AluOpType.add,
                        ).then_inc(s_dve, 1)
                    if 0 <= u - 4 < nt:
                        t = u - 4
                        vector.wait_ge(s_act, done[("sqrt", t)])
                        if t >= DS:
                            vector.wait_ge(s_pl, done[("prescale", t - DS)])
                        nc.vector.reciprocal(inv[t % DS][:], rms[t % DS][:]).then_inc(
                            s_dve, 1
                        )
                    if 0 <= u - 7 < nt:
                        t = u - 7
                        vector.wait_ge(s_pe, done[("fwdT", t)])
                        if t >= DXT:
                            vector.wait_ge(s_pe, done[("mms", t - DXT)])
                        nc.vector.tensor_copy(
                            xTs[t % DXT][:], ps1[t % DP1][:]
                        ).then_inc(s_dve, 1)

            @block.gpsimd
            def _(pool):
                pool.wait_ge(s_c, CONST_TARGET)
                for u in range(nt + 15):
                    if 0 <= u - 5 < nt:
                        t = u - 5
                        pool.wait_ge(s_dve, done[("recip", t)])
                        if t >= DXS:
                            pool.wait_ge(s_pe, done[("fwdT", t - DXS)])
                        nc.gpsimd.tensor_mul(
                            g16(xs[t % DXS][:]),
                            g16(xin[t % DXI][:]),
                            inv[t % DS][:].unsqueeze(-1).broadcast_to([128, 16, DE]),
                        ).then_inc(s_pl, 1)
                    if u >= 13 and (u - 13) % 4 == 0 and (u - 13) // 4 < ng:
                        g = (u - 13) // 4
                        pool.wait_ge(s_act, done[("final", 4 * g + 3)])
                        half = 4096 * ((4 * g) % DO // 4)
                        pool.dma_start(
                            out=out_dst(g), in_=outsb[:, half : half + 4096]
                        ).then_inc(s_od[g % 2], 16)

            @block.tensor
            def _(tensor):
                tensor.wait_ge(s_c, CONST_TARGET)
                for u in range(nt + 9):
                    tT = u - 6
                    tM = u - 8
                    doT = 0 <= tT < nt
                    doM = 0 <= tM < nt
                    inc = done[("peiter", u)]
                    if doT:
                        tensor.wait_ge(s_pl, done[("prescale", tT)])
                        if tT >= DP1:
                            tensor.wait_ge(s_dve, done[("copy1", tT - DP1)])
                        for c in range(8):
                            mi = nc.tensor.matmul(
                                ps1[tT % DP1][:, 128 * c : 128 * c + 128],
                                xs[tT % DXS][:, 128 * c : 128 * c + 128],
                                idb[:],
                                start=True, stop=True, skip_group_check=True,
                            )
                            if c == 7 and not doM:
                                mi.then_inc(s_pe, inc)
                    if doM:
                        if tM >= DP2:
                            tensor.wait_ge(s_act, done[("final", tM - DP2)])
                        pt = ps2[tM % DP2]
                        rp = (
                            rpm[:, 64 * tM : 64 * tM + 64]
                            .unsqueeze(1)
                            .broadcast_to([4, 8, DE])
                        )
                        cpm = cpm0 if tM < nt // 2 else cpm1
                        for h in range(2):
                            nc.tensor.matmul(
                                pt[:, 512 * h : 512 * h + 512], i4[:], rp,
                                start=True, stop=False, skip_group_check=True,
                            )
                        for h in range(2):
                            nc.tensor.matmul(
                                pt[:, 512 * h : 512 * h + 512],
                                i32[:],
                                cpm[:, 512 * h : 512 * h + 512],
                                start=False, stop=False, skip_group_check=True,
                            )
                        tensor.wait_ge(s_dve, done[("copy1", tM)])
                        for c in range(8):
                            mi = nc.tensor.matmul(
                                pt[:, 128 * c : 128 * c + 128],
                                xTs[tM % DXT][:, 128 * c : 128 * c + 128],
                                wg[:],
                                start=False, stop=True, skip_group_check=True,
                            )
                            if c == 7:
                                mi.then_inc(s_pe, inc)

    return nc


_NC_CACHE = {}


def _get_nc():
    if "nc" not in _NC_CACHE:
        _NC_CACHE["nc"] = _build_nc()
    return _NC_CACHE["nc"]


def _make_in_maps(edges, nodes, g_node, g_edge, W, b):
    import ml_dtypes

    bf = ml_dtypes.bfloat16
    edges = np.ascontiguousarray(edges, dtype=np.float32)
    nodes = np.ascontiguousarray(nodes, dtype=np.float32)

    # ---- host: tiny node path (B*N*dn = 131K elems)
    ms = np.mean(np.square(nodes), axis=-1, keepdims=True)
    nodes_n = nodes / np.sqrt(ms + EPS) * g_node  # [B, N, 128]
    Wr, Wc, We = W[:DN], W[DN : 2 * DN], W[2 * DN :]
    row_proj = (nodes_n @ Wr).astype(np.float32)  # [B, N, 64]
    col_proj = (nodes_n @ Wc).astype(np.float32)  # [B, N, 64]
    Wg = (g_edge[:, None] * We).astype(np.float32)  # fold g_edge into We

    # block-diagonal Wg (even-r rows top-left, odd-r rows bottom-right)
    wgblk = np.zeros((128, 128), dtype=np.float32)
    wgblk[:64, :64] = Wg
    wgblk[64:, 64:] = Wg
    wgblk = wgblk.astype(bf)
    idb = np.eye(128, dtype=np.float32).astype(bf)
    i4 = np.kron(np.eye(4, dtype=np.float32), np.ones((1, 32), np.float32)).astype(bf)
    i32 = np.tile(np.eye(32, dtype=np.float32), (1, 4)).astype(bf)

    # col_proj + bias, per b: cpm_b[p', 64r + e'] = cp_b[16 p' + r, e']
    cp = (col_proj + b).astype(np.float32)  # [B, 512, 64]
    cpm0 = np.ascontiguousarray(cp[0].reshape(32, 1024)).astype(bf)
    cpm1 = np.ascontiguousarray(cp[1].reshape(32, 1024)).astype(bf)

    in_maps = []
    for c in range(NCORES):
        xc = edges[:, c * IPC : (c + 1) * IPC]  # [B, 64, 512, 64]
        xc = np.ascontiguousarray(xc).reshape(NBLK, N, DE)
        # rpm[k, 64T + e'] = row_proj[block 4T + k][e']
        rp = row_proj[:, c * IPC : (c + 1) * IPC].reshape(NBLK, DE)  # [128, 64]
        rpm = np.ascontiguousarray(
            rp.reshape(NT, 4, DE).transpose(1, 0, 2).reshape(4, NT * DE)
        ).astype(bf)
        in_maps.append(
            {
                "x": xc,
                "wg": wgblk,
                "idb": idb,
                "i4": i4,
                "i32": i32,
                "rpm": rpm,
                "cpm0": cpm0,
                "cpm1": cpm1,
            }
        )
    return in_maps


def _run(edges, nodes, g_node, g_edge, W, b, trace=False, **spmd_kwargs):
    in_maps = _make_in_maps(edges, nodes, g_node, g_edge, W, b)

    from concourse.bass_utils import run_bass_kernel_spmd

    nc = _get_nc()
    res = run_bass_kernel_spmd(
        nc, in_maps, list(range(NCORES)), trace=trace, **spmd_kwargs
    )

    out = np.empty((B, N, N, DE), dtype=np.float32)
    for c in range(NCORES):
        oc = res.results[c]["out"].reshape(B, IPC, N, DE)
        out[:, c * IPC : (c + 1) * IPC] = oc
    return out, res


def kernel(edges, nodes, g_node, g_edge, W, b):
    return _run(edges, nodes, g_node, g_edge, W, b)[0]


if __name__ == "__main__":
    rng = np.random.default_rng(0)
    edges = rng.standard_normal((B, N, N, DE), dtype=np.float32)
    nodes = rng.standard_normal((B, N, DN), dtype=np.float32)
    g_node = np.ones(DN, np.float32)
    g_edge = np.ones(DE, np.float32)
    W = rng.standard_normal((2 * DN + DE, DE), dtype=np.float32) / 18.0
    b = (rng.standard_normal(DE) * 0.01).astype(np.float32)
    o = kernel(edges, nodes, g_node, g_edge, W, b)
    print(o.shape, o.dtype)


# revision 18
# speedup vs baseline: 1.3381x; 1.1230x over previous
"""Bass/Trainium2 kernel for nn_NodesToEdges (gnn_message_passing).

out[b,i,j,:] = rms(edges[b,i,j,:])*g_e @ We + rms(nodes[b,i,:])*g_n @ Wr
             + rms(nodes[b,j,:])*g_n @ Wc + bias

Strategy: shard over i (rows) across 8 cores. The tiny node path
(row_proj / col_proj) is precomputed on host. The edge path (134 MB in /
134 MB out, memory-bound) runs on device.

Per core: 128 blocks (b,i) of [512 j, 64 e], processed as 32 TILES of 4
blocks = [128 part = (a,p=j//16), 1024 free = (r=j%16, e)] (4 KB DMA
descriptors). Per tile:
  ACT: square -> DVE: 16-group reduce -> ACT: sqrt(mean+eps) ->
  DVE: recip -> Pool: prescale (x*inv -> bf16) ->
  PE: 8x transpose (128-col chunks, bf16) -> DVE: copy psum->sbuf ->
  PE: row/col adds via indicator-stationary matmuls + 8 chunk-matmuls
      with the TRANSPOSED DATA as stationary (output lands row-major,
      f32 psum; no back-transpose) ->
  ACT: copy psum->sbuf -> Pool: SWDGE out-DMA (4-tile batches).
In-DMA on the SP HWDGE ring; out-DMA on the Pool SWDGE ring.
"""

import numpy as np

B, N, DE, DN = 2, 512, 64, 128
NCORES = 8
IPC = N // NCORES          # 64 i-rows per core
NBLK = B * IPC             # 128 blocks of [512, 64] per core
NT = NBLK // 4             # 32 tiles of 4 blocks
EPS = float(np.finfo(np.float32).eps)

# buffer depths (slots)
DXI = 6    # xin tiles in flight
DQ = 3     # sq slots
DS = 3     # ss / rms / inv slots
DXS = 3    # xs (prescaled bf16)
DP1 = 2    # psum transpose slots ([128,1024] f32, 2 banks each)
DXT = 3    # xTs (sbuf bf16)
DP2 = 2    # psum matmul banks (2 banks each)
DO = 8     # outsb slots (two 4-tile out-DMA batches)


def _build_nc(nt=NT):
    from contextlib import ExitStack

    import concourse.bass as bass
    import concourse.mybir as mybir

    f32 = mybir.dt.float32
    bf16 = mybir.dt.bfloat16
    SQRT = mybir.ActivationFunctionType.Sqrt

    nc = bass.Bass()
    nblk = 4 * nt
    x_d = nc.declare_dram_parameter("x", [nblk, N, DE], f32, isOutput=False)
    wg_d = nc.declare_dram_parameter("wg", [128, 128], bf16, isOutput=False)
    idb_d = nc.declare_dram_parameter("idb", [128, 128], bf16, isOutput=False)
    i4_d = nc.declare_dram_parameter("i4", [128, 128], bf16, isOutput=False)
    i32_d = nc.declare_dram_parameter("i32", [128, 128], bf16, isOutput=False)
    rpm_d = nc.declare_dram_parameter("rpm", [128, 64 * nt], bf16, isOutput=False)
    cpm0_d = nc.declare_dram_parameter("cpm0", [128, 1024], bf16, isOutput=False)
    cpm1_d = nc.declare_dram_parameter("cpm1", [128, 1024], bf16, isOutput=False)
    out_d = nc.declare_dram_parameter("out", [nblk, N, DE], f32, isOutput=True)

    st = ExitStack()
    with st:
        sb = lambda name, shape, dt: st.enter_context(nc.sbuf_tensor(name, shape, dt))
        wg = sb("wg_sb", [128, 128], bf16)
        idb = sb("idb_sb", [128, 128], bf16)
        i4 = sb("i4_sb", [128, 128], bf16)
        i32 = sb("i32_sb", [128, 128], bf16)
        rpm = sb("rpm_sb", [128, 64 * nt], bf16)
        cpm0 = sb("cpm0_sb", [128, 1024], bf16)
        cpm1 = sb("cpm1_sb", [128, 1024], bf16)
        epsb = sb("epsb", [128, 1], f32)
        xin = [sb(f"xin{i}", [128, 1024], f32) for i in range(DXI)]
        sq = [sb(f"sq{i}", [128, 1024], bf16) for i in range(DQ)]
        ss = [sb(f"ss{i}", [128, 16], f32) for i in range(DS)]
        rms = [sb(f"rms{i}", [128, 16], f32) for i in range(DS)]
        inv = [sb(f"inv{i}", [128, 16], f32) for i in range(DS)]
        xs = [sb(f"xs{i}", [128, 1024], bf16) for i in range(DXS)]
        xTs = [sb(f"xTs{i}", [128, 1024], bf16) for i in range(DXT)]
        outsb = sb("outsb", [128, DO * 1024], f32)
        ps1 = [
            st.enter_context(nc.psum_tensor(f"ps1{i}", [128, 1024], f32))
            for i in range(DP1)
        ]
        ps2 = [
            st.enter_context(nc.psum_tensor(f"ps2{i}", [128, 1024], f32))
            for i in range(DP2)
        ]

        sem = lambda name: st.enter_context(nc.semaphore(name))
        s_c = sem("s_c")
        s_in = [sem(f"s_in{i}") for i in range(DXI)]
        s_od = [sem(f"s_od{i}") for i in range(2)]
        s_act = sem("s_act")
        s_dve = sem("s_dve")
        s_pl = sem("s_pl")
        s_pe = sem("s_pe")

        # --- precomputed semaphore values: done[(op, t)] = sem value once
        # op(t) has retired.
        done = {}
        for t in range(nt):
            done[("in", t)] = 16 * (t // DXI + 1)      # on s_in[t % DXI]
        ng = nt // 4
        for g in range(ng):
            done[("outd", g)] = 16 * (g // 2 + 1)      # on s_od[g % 2]

        # ACT iter u: square(u-1), sqrt(u-3), final(u-9)
        c = 0
        for u in range(nt + 11):
            if 0 <= u - 1 < nt:
                c += 1
                done[("square", u - 1)] = c
            if 0 <= u - 3 < nt:
                c += 1
                done[("sqrt", u - 3)] = c
            if 0 <= u - 9 < nt:
                c += 1
                done[("final", u - 9)] = c
        # DVE iter u: reduce(u-2), recip(u-4), copy1(u-7)
        c = 0
        for u in range(nt + 8):
            if 0 <= u - 2 < nt:
                c += 1
                done[("reduce", u - 2)] = c
            if 0 <= u - 4 < nt:
                c += 1
                done[("recip", u - 4)] = c
            if 0 <= u - 7 < nt:
                c += 1
                done[("copy1", u - 7)] = c
        # Pool iter u: prescale(u-5)
        c = 0
        for u in range(nt + 6):
            if 0 <= u - 5 < nt:
                c += 1
                done[("prescale", u - 5)] = c
        # PE iter u: fwdT(u-6) x8 normal-mode identity matmuls, then
        # row/col + chunk-mms(u-8) x12. ONE s_pe inc per iteration on the
        # very last matmul -- mid-stream sem updates stall the next matmul
        # until the updating op retires, so expose none.
        c = 0
        for u in range(nt + 9):
            k = 0
            if 0 <= u - 6 < nt:
                k += 8
            if 0 <= u - 8 < nt:
                k += 12
            c += k
            if 0 <= u - 6 < nt:
                done[("fwdT", u - 6)] = c
            if 0 <= u - 8 < nt:
                done[("mms", u - 8)] = c
            done[("peiter", u)] = k

        CONST_TARGET = 7 * 16

        def in_src(t):
            return x_d[4 * t : 4 * t + 4].rearrange(
                "a (p r) e -> (a p) (r e)", p=32, r=16
            )

        def out_dst(g):
            return out_d[16 * g : 16 * g + 16].rearrange(
                "(T a) (p r) e -> (a p) T (r e)", T=4, a=4, p=32, r=16
            )

        def g16(x):
            return x.rearrange("p (g e) -> p g e", e=DE)

        with nc.Block() as block:

            @block.sync
            def _(sync):
                for cdst, csrc in (
                    (wg, wg_d),
                    (idb, idb_d),
                    (i4, i4_d),
                    (i32, i32_d),
                    (rpm, rpm_d),
                    (cpm0, cpm0_d),
                    (cpm1, cpm1_d),
                ):
                    sync.dma_start(out=cdst[:], in_=csrc[:]).then_inc(s_c, 16)
                for t in range(nt):
                    if t >= DXI:
                        sync.wait_ge(s_pl, done[("prescale", t - DXI)])
                    sync.dma_start(out=xin[t % DXI][:], in_=in_src(t)).then_inc(
                        s_in[t % DXI], 16
                    )

            @block.scalar
            def _(scalar):
                scalar.wait_ge(s_c, CONST_TARGET)
                for u in range(nt + 11):
                    if 0 <= u - 1 < nt:
                        t = u - 1
                        scalar.wait_ge(s_in[t % DXI], done[("in", t)])
                        if t >= DQ:
                            scalar.wait_ge(s_dve, done[("reduce", t - DQ)])
                        nc.scalar.square(
                            sq[t % DQ][:], xin[t % DXI][:].bitcast(bf16)[:, 1::2]
                        ).then_inc(s_act, 1)
                    if 0 <= u - 3 < nt:
                        t = u - 3
                        scalar.wait_ge(s_dve, done[("reduce", t)])
                        if t >= DS:
                            scalar.wait_ge(s_dve, done[("recip", t - DS)])
                        nc.scalar.activation(
                            rms[t % DS][:], ss[t % DS][:], SQRT,
                            bias=epsb[:], scale=1.0 / DE,
                        ).then_inc(s_act, 1)
                    if 0 <= u - 9 < nt:
                        t = u - 9
                        scalar.wait_ge(s_pe, done[("mms", t)])
                        if t >= DO:
                            gp = (t - DO) // 4
                            scalar.wait_ge(s_od[gp % 2], done[("outd", gp)])
                        nc.scalar.copy(
                            outsb[:, 1024 * (t % DO) : 1024 * (t % DO) + 1024],
                            ps2[t % DP2][:],
                        ).then_inc(s_act, 1)

            @block.vector
            def _(vector):
                vector.wait_ge(s_c, CONST_TARGET)
                nc.vector.memset(epsb[:], EPS)
                for u in range(nt + 8):
                    if 0 <= u - 2 < nt:
                        t = u - 2
                        vector.wait_ge(s_act, done[("square", t)])
                        if t >= DS:
                            vector.wait_ge(s_act, done[("sqrt", t - DS)])
                        nc.vector.tensor_reduce(
                            ss[t % DS][:],
                            g16(sq[t % DQ][:]),
                            axis=mybir.AxisListType.X,
                            op=mybir.AluOpType.add,
                        ).then_inc(s_dve, 1)
                    if 0 <= u - 4 < nt:
                        t = u - 4
                        vector.wait_ge(s_act, done[("sqrt", t)])
                        if t >= DS:
                            vector.wait_ge(s_pl, done[("prescale", t - DS)])
                        nc.vector.reciprocal(inv[t % DS][:], rms[t % DS][:]).then_inc(
                            s_dve, 1
                        )
                    if 0 <= u - 7 < nt:
                        t = u - 7
                        vector.wait_ge(s_pe, done[("fwdT", t)])
                        if t >= DXT:
                            vector.wait_ge(s_pe, done[("mms", t - DXT)])
                        nc.vector.tensor_copy(
                            xTs[t % DXT][:], ps1[t % DP1][:]
                        ).then_inc(s_dve, 1)

            @block.gpsimd
            def _(pool):
                pool.wait_ge(s_c, CONST_TARGET)
                for u in range(nt + 15):
                    if 0 <= u - 5 < nt:
                        t = u - 5
                        pool.wait_ge(s_dve, done[("recip", t)])
                        if t >= DXS:
                            pool.wait_ge(s_pe, done[("fwdT", t - DXS)])
                        nc.gpsimd.tensor_mul(
                            g16(xs[t % DXS][:]),
                            g16(xin[t % DXI][:]),
                            inv[t % DS][:].unsqueeze(-1).broadcast_to([128, 16, DE]),
                        ).then_inc(s_pl, 1)
                    if u >= 13 and (u - 13) % 4 == 0 and (u - 13) // 4 < ng:
                        g = (u - 13) // 4
                        pool.wait_ge(s_act, done[("final", 4 * g + 3)])
                        half = 4096 * ((4 * g) % DO // 4)
                        pool.dma_start(
                            out=out_dst(g), in_=outsb[:, half : half + 4096]
                        ).then_inc(s_od[g % 2], 16)

            @block.tensor
            def _(tensor):
                tensor.wait_ge(s_c, CONST_TARGET)
                for u in range(nt + 9):
                    tT = u - 6
                    tM = u - 8
                    doT = 0 <= tT < nt
                    doM = 0 <= tM < nt
                    inc = done[("peiter", u)]
                    if doT:
                        tensor.wait_ge(s_pl, done[("prescale", tT)])
                        if tT >= DP1:
                            tensor.wait_ge(s_dve, done[("copy1", tT - DP1)])
                        for c in range(8):
                            mi = nc.tensor.matmul(
                                ps1[tT % DP1][:, 128 * c : 128 * c + 128],
                                xs[tT % DXS][:, 128 * c : 128 * c + 128],
                                idb[:],
                                start=True, stop=True, skip_group_check=True,
                            )
                            if c == 7 and not doM:
                                mi.then_inc(s_pe, inc)
                    if doM:
                        if tM >= DP2:
                            tensor.wait_ge(s_act, done[("final", tM - DP2)])
                        pt = ps2[tM % DP2]
                        rp = (
                            rpm[:, 64 * tM : 64 * tM + 64]
                            .unsqueeze(1)
                            .broadcast_to([128, 8, DE])
                        )
                        cpm = cpm0 if tM < nt // 2 else cpm1
                        for h in range(2):
                            nc.tensor.matmul(
                                pt[:, 512 * h : 512 * h + 512], i4[:], rp,
                                start=True, stop=False, skip_group_check=True,
                            )
                        for h in range(2):
                            nc.tensor.matmul(
                                pt[:, 512 * h : 512 * h + 512],
                                i32[:],
                                cpm[:, 512 * h : 512 * h + 512],
                                start=False, stop=False, skip_group_check=True,
                            )
                        tensor.wait_ge(s_dve, done[("copy1", tM)])
                        for c in range(8):
                            mi = nc.tensor.matmul(
                                pt[:, 128 * c : 128 * c + 128],
                                xTs[tM % DXT][:, 128 * c : 128 * c + 128],
                                wg[:],
                                start=False, stop=True, skip_group_check=True,
                            )
                            if c == 7:
                                mi.then_inc(s_pe, inc)

    return nc


_NC_CACHE = {}


def _get_nc():
    if "nc" not in _NC_CACHE:
        _NC_CACHE["nc"] = _build_nc()
    return _NC_CACHE["nc"]


def _make_in_maps(edges, nodes, g_node, g_edge, W, b):
    import ml_dtypes

    bf = ml_dtypes.bfloat16
    edges = np.ascontiguousarray(edges, dtype=np.float32)
    nodes = np.ascontiguousarray(nodes, dtype=np.float32)

    # ---- host: tiny node path (B*N*dn = 131K elems)
    ms = np.mean(np.square(nodes), axis=-1, keepdims=True)
    nodes_n = nodes / np.sqrt(ms + EPS) * g_node  # [B, N, 128]
    Wr, Wc, We = W[:DN], W[DN : 2 * DN], W[2 * DN :]
    row_proj = (nodes_n @ Wr).astype(np.float32)  # [B, N, 64]
    col_proj = (nodes_n @ Wc).astype(np.float32)  # [B, N, 64]
    Wg = (g_edge[:, None] * We).astype(np.float32)  # fold g_edge into We

    # block-diagonal Wg (even-r rows top-left, odd-r rows bottom-right)
    wgblk = np.zeros((128, 128), dtype=np.float32)
    wgblk[:64, :64] = Wg
    wgblk[64:, 64:] = Wg
    wgblk = wgblk.astype(bf)
    idb = np.eye(128, dtype=np.float32).astype(bf)
    i4 = np.zeros((128, 128), np.float32)
    i4[:4] = np.kron(np.eye(4, dtype=np.float32), np.ones((1, 32), np.float32))
    i4 = i4.astype(bf)
    i32 = np.zeros((128, 128), np.float32)
    i32[:32] = np.tile(np.eye(32, dtype=np.float32), (1, 4))
    i32 = i32.astype(bf)

    # col_proj + bias, per b: cpm_b[p', 64r + e'] = cp_b[16 p' + r, e']
    cp = (col_proj + b).astype(np.float32)  # [B, 512, 64]
    cpm0 = np.zeros((128, 1024), np.float32)
    cpm0[:32] = cp[0].reshape(32, 1024)
    cpm0 = cpm0.astype(bf)
    cpm1 = np.zeros((128, 1024), np.float32)
    cpm1[:32] = cp[1].reshape(32, 1024)
    cpm1 = cpm1.astype(bf)

    in_maps = []
    for c in range(NCORES):
        xc = edges[:, c * IPC : (c + 1) * IPC]  # [B, 64, 512, 64]
        xc = np.ascontiguousarray(xc).reshape(NBLK, N, DE)
        # rpm[k, 64T + e'] = row_proj[block 4T + k][e']
        rp = row_proj[:, c * IPC : (c + 1) * IPC].reshape(NBLK, DE)  # [128, 64]
        rpm = np.zeros((128, NT * DE), np.float32)
        rpm[:4] = rp.reshape(NT, 4, DE).transpose(1, 0, 2).reshape(4, NT * DE)
        rpm = rpm.astype(bf)
        in_maps.append(
            {
                "x": xc,
                "wg": wgblk,
                "idb": idb,
                "i4": i4,
                "i32": i32,
                "rpm": rpm,
                "cpm0": cpm0,
                "cpm1": cpm1,
            }
        )
    return in_maps


def _run(edges, nodes, g_node, g_edge, W, b, trace=False, **spmd_kwargs):
    in_maps = _make_in_maps(edges, nodes, g_node, g_edge, W, b)

    from concourse.bass_utils import run_bass_kernel_spmd

    nc = _get_nc()
    res = run_bass_kernel_spmd(
        nc, in_maps, list(range(NCORES)), trace=trace, **spmd_kwargs
    )

    out = np.empty((B, N, N, DE), dtype=np.float32)
    for c in range(NCORES):
        oc = res.results[c]["out"].reshape(B, IPC, N, DE)
        out[:, c * IPC : (c + 1) * IPC] = oc
    return out, res


def kernel(edges, nodes, g_node, g_edge, W, b):
    return _run(edges, nodes, g_node, g_edge, W, b)[0]


if __name__ == "__main__":
    rng = np.random.default_rng(0)
    edges = rng.standard_normal((B, N, N, DE), dtype=np.float32)
    nodes = rng.standard_normal((B, N, DN), dtype=np.float32)
    g_node = np.ones(DN, np.float32)
    g_edge = np.ones(DE, np.float32)
    W = rng.standard_normal((2 * DN + DE, DE), dtype=np.float32) / 18.0
    b = (rng.standard_normal(DE) * 0.01).astype(np.float32)
    o = kernel(edges, nodes, g_node, g_edge, W, b)
    print(o.shape, o.dtype)


# revision 19
# speedup vs baseline: 1.4943x; 1.1167x over previous
"""Bass/Trainium2 kernel for nn_NodesToEdges (gnn_message_passing).

out[b,i,j,:] = rms(edges[b,i,j,:])*g_e @ We + rms(nodes[b,i,:])*g_n @ Wr
             + rms(nodes[b,j,:])*g_n @ Wc + bias

Strategy: shard over i (rows) across 8 cores. The tiny node path
(row_proj / col_proj) is precomputed on host. The edge path (134 MB in /
134 MB out, memory-bound) runs on device.

Per core: 128 blocks (b,i) of [512 j, 64 e], processed as 32 TILES of 4
blocks = [128 part = (a,p=j//16), 1024 free = (r=j%16, e)] (4 KB DMA
descriptors). Per tile:
  ACT: square -> DVE: 16-group reduce -> ACT: sqrt(mean+eps) ->
  DVE: recip -> Pool: prescale (x*inv -> bf16) ->
  PE: 8x transpose (128-col chunks, bf16) -> DVE: copy psum->sbuf ->
  PE: row/col adds via indicator-stationary matmuls + 8 chunk-matmuls
      with the TRANSPOSED DATA as stationary (output lands row-major,
      f32 psum; no back-transpose) ->
  ACT: copy psum->sbuf -> Pool: SWDGE out-DMA (4-tile batches).
In-DMA on the SP HWDGE ring; out-DMA on the Pool SWDGE ring.
"""

import numpy as np

B, N, DE, DN = 2, 512, 64, 128
NCORES = 8
IPC = N // NCORES          # 64 i-rows per core
NBLK = B * IPC             # 128 blocks of [512, 64] per core
NT = NBLK // 4             # 32 tiles of 4 blocks
EPS = float(np.finfo(np.float32).eps)

# buffer depths (slots)
DXI = 6    # xin tiles in flight
DQ = 3     # sq slots
DS = 3     # ss / rms / inv slots
DXS = 3    # xs (prescaled bf16)
DP1 = 2    # psum transpose slots ([128,1024] f32, 2 banks each)
DXT = 3    # xTs (sbuf bf16)
DP2 = 3    # psum matmul slots (2 banks each)
DO = 8     # outsb slots (two 4-tile out-DMA batches)


def _build_nc(nt=NT):
    from contextlib import ExitStack

    import concourse.bass as bass
    import concourse.mybir as mybir

    f32 = mybir.dt.float32
    bf16 = mybir.dt.bfloat16
    SQRT = mybir.ActivationFunctionType.Sqrt

    nc = bass.Bass()
    nblk = 4 * nt
    x_d = nc.declare_dram_parameter("x", [nblk, N, DE], f32, isOutput=False)
    wg_d = nc.declare_dram_parameter("wg", [128, 128], bf16, isOutput=False)
    idb_d = nc.declare_dram_parameter("idb", [128, 128], bf16, isOutput=False)
    i4_d = nc.declare_dram_parameter("i4", [128, 128], bf16, isOutput=False)
    i32_d = nc.declare_dram_parameter("i32", [128, 128], bf16, isOutput=False)
    rpm_d = nc.declare_dram_parameter("rpm", [128, 64 * nt], bf16, isOutput=False)
    cpm0_d = nc.declare_dram_parameter("cpm0", [128, 1024], bf16, isOutput=False)
    cpm1_d = nc.declare_dram_parameter("cpm1", [128, 1024], bf16, isOutput=False)
    out_d = nc.declare_dram_parameter("out", [nblk, N, DE], f32, isOutput=True)

    st = ExitStack()
    with st:
        sb = lambda name, shape, dt: st.enter_context(nc.sbuf_tensor(name, shape, dt))
        wg = sb("wg_sb", [128, 128], bf16)
        idb = sb("idb_sb", [128, 128], bf16)
        i4 = sb("i4_sb", [128, 128], bf16)
        i32 = sb("i32_sb", [128, 128], bf16)
        rpm = sb("rpm_sb", [128, 64 * nt], bf16)
        cpm0 = sb("cpm0_sb", [128, 1024], bf16)
        cpm1 = sb("cpm1_sb", [128, 1024], bf16)
        epsb = sb("epsb", [128, 1], f32)
        xin = [sb(f"xin{i}", [128, 1024], f32) for i in range(DXI)]
        sq = [sb(f"sq{i}", [128, 1024], bf16) for i in range(DQ)]
        ss = [sb(f"ss{i}", [128, 16], f32) for i in range(DS)]
        inv = [sb(f"inv{i}", [128, 16], f32) for i in range(DS)]
        xs = [sb(f"xs{i}", [128, 1024], bf16) for i in range(DXS)]
        xTs = [sb(f"xTs{i}", [128, 1024], bf16) for i in range(DXT)]
        outsb = sb("outsb", [128, DO * 1024], f32)
        ps1 = [
            st.enter_context(nc.psum_tensor(f"ps1{i}", [128, 1024], bf16))
            for i in range(DP1)
        ]
        ps2 = [
            st.enter_context(nc.psum_tensor(f"ps2{i}", [128, 1024], f32))
            for i in range(DP2)
        ]

        sem = lambda name: st.enter_context(nc.semaphore(name))
        s_c = sem("s_c")
        s_in = [sem(f"s_in{i}") for i in range(DXI)]
        s_od = [sem(f"s_od{i}") for i in range(2)]
        s_act = sem("s_act")
        s_dve = sem("s_dve")
        s_pl = sem("s_pl")
        s_pe = sem("s_pe")

        # --- precomputed semaphore values: done[(op, t)] = sem value once
        # op(t) has retired.
        done = {}
        for t in range(nt):
            done[("in", t)] = 16 * (t // DXI + 1)      # on s_in[t % DXI]
        ng = nt // 4
        for g in range(ng):
            done[("outd", g)] = 16 * (g // 2 + 1)      # on s_od[g % 2]

        # Stage map: in@t, square@ACT[t+1], reduce@DVE[t+2], rsqrt@ACT[t+3],
        # prescale@Pool[t+4], fwdT@PE[t+5], copy1@DVE[t+6], mms@PE[t+7],
        # final@ACT[t+8], outdma@Pool[4g+12].
        # ACT iter u: square(u-1), rsqrt(u-3), final(u-8)
        c = 0
        for u in range(nt + 9):
            if 0 <= u - 1 < nt:
                c += 1
                done[("square", u - 1)] = c
            if 0 <= u - 3 < nt:
                c += 1
                done[("rsqrt", u - 3)] = c
            if 0 <= u - 8 < nt:
                c += 1
                done[("final", u - 8)] = c
        # DVE iter u: reduce(u-2), copy1(u-6)
        c = 0
        for u in range(nt + 7):
            if 0 <= u - 2 < nt:
                c += 1
                done[("reduce", u - 2)] = c
            if 0 <= u - 6 < nt:
                c += 1
                done[("copy1", u - 6)] = c
        # Pool iter u: prescale(u-4)
        c = 0
        for u in range(nt + 5):
            if 0 <= u - 4 < nt:
                c += 1
                done[("prescale", u - 4)] = c
        # PE iter u: fwdT(u-5) x8, then mms(u-7) x12; one s_pe inc per
        # iteration on the very last matmul (mid-stream sem updates stall
        # the next matmul until the updating op retires).
        c = 0
        for u in range(nt + 8):
            k = 0
            if 0 <= u - 5 < nt:
                k += 8
            if 0 <= u - 7 < nt:
                k += 12
            c += k
            if 0 <= u - 5 < nt:
                done[("fwdT", u - 5)] = c
            if 0 <= u - 7 < nt:
                done[("mms", u - 7)] = c
            done[("peiter", u)] = k

        CONST_TARGET = 7 * 16

        def in_src(t):
            return x_d[4 * t : 4 * t + 4].rearrange(
                "a (p r) e -> (a p) (r e)", p=32, r=16
            )

        def out_dst(g):
            return out_d[16 * g : 16 * g + 16].rearrange(
                "(T a) (p r) e -> (a p) T (r e)", T=4, a=4, p=32, r=16
            )

        def g16(x):
            return x.rearrange("p (g e) -> p g e", e=DE)

        with nc.Block() as block:

            @block.sync
            def _(sync):
                for cdst, csrc in (
                    (wg, wg_d),
                    (idb, idb_d),
                    (i4, i4_d),
                    (i32, i32_d),
                    (rpm, rpm_d),
                    (cpm0, cpm0_d),
                    (cpm1, cpm1_d),
                ):
                    sync.dma_start(out=cdst[:], in_=csrc[:]).then_inc(s_c, 16)
                for t in range(nt):
                    if t >= DXI:
                        sync.wait_ge(s_pl, done[("prescale", t - DXI)])
                    sync.dma_start(out=xin[t % DXI][:], in_=in_src(t)).then_inc(
                        s_in[t % DXI], 16
                    )

            def rsqrt_act(out, in_, bias, scale):
                eng = nc.scalar
                ins = [
                    eng.lower_ap(in_),
                    eng.lower_ap(bias),
                    mybir.ImmediateValue(dtype=f32, value=float(scale)),
                    mybir.ImmediateValue(dtype=f32, value=0.0),
                ]
                return eng.add_instruction(
                    mybir.InstActivation(
                        name=nc.get_next_instruction_name(),
                        func=mybir.ActivationFunctionType.Rsqrt,
                        ins=ins,
                        outs=[eng.lower_ap(out)],
                    )
                )

            @block.scalar
            def _(scalar):
                scalar.wait_ge(s_c, CONST_TARGET)
                for u in range(nt + 9):
                    if 0 <= u - 1 < nt:
                        t = u - 1
                        scalar.wait_ge(s_in[t % DXI], done[("in", t)])
                        if t >= DQ:
                            scalar.wait_ge(s_dve, done[("reduce", t - DQ)])
                        nc.scalar.square(
                            sq[t % DQ][:], xin[t % DXI][:].bitcast(bf16)[:, 1::2]
                        ).then_inc(s_act, 1)
                    if 0 <= u - 3 < nt:
                        t = u - 3
                        scalar.wait_ge(s_dve, done[("reduce", t)])
                        if t >= DS:
                            scalar.wait_ge(s_pl, done[("prescale", t - DS)])
                        rsqrt_act(
                            inv[t % DS][:], ss[t % DS][:], epsb[:], 1.0 / DE
                        ).then_inc(s_act, 1)
                    if 0 <= u - 8 < nt:
                        t = u - 8
                        scalar.wait_ge(s_pe, done[("mms", t)])
                        if t >= DO:
                            gp = (t - DO) // 4
                            scalar.wait_ge(s_od[gp % 2], done[("outd", gp)])
                        nc.scalar.copy(
                            outsb[:, 1024 * (t % DO) : 1024 * (t % DO) + 1024],
                            ps2[t % DP2][:],
                        ).then_inc(s_act, 1)

            @block.vector
            def _(vector):
                vector.wait_ge(s_c, CONST_TARGET)
                nc.vector.memset(epsb[:], EPS)
                for u in range(nt + 7):
                    if 0 <= u - 2 < nt:
                        t = u - 2
                        vector.wait_ge(s_act, done[("square", t)])
                        if t >= DS:
                            vector.wait_ge(s_act, done[("rsqrt", t - DS)])
                        nc.vector.tensor_reduce(
                            ss[t % DS][:],
                            g16(sq[t % DQ][:]),
                            axis=mybir.AxisListType.X,
                            op=mybir.AluOpType.add,
                        ).then_inc(s_dve, 1)
                    if 0 <= u - 6 < nt:
                        t = u - 6
                        vector.wait_ge(s_pe, done[("fwdT", t)])
                        if t >= DXT:
                            vector.wait_ge(s_pe, done[("mms", t - DXT)])
                        nc.vector.tensor_copy(
                            xTs[t % DXT][:], ps1[t % DP1][:]
                        ).then_inc(s_dve, 1)

            @block.gpsimd
            def _(pool):
                pool.wait_ge(s_c, CONST_TARGET)
                for u in range(nt + 13):
                    if 0 <= u - 4 < nt:
                        t = u - 4
                        pool.wait_ge(s_act, done[("rsqrt", t)])
                        if t >= DXS:
                            pool.wait_ge(s_pe, done[("fwdT", t - DXS)])
                        nc.gpsimd.tensor_mul(
                            g16(xs[t % DXS][:]),
                            g16(xin[t % DXI][:]),
                            inv[t % DS][:].unsqueeze(-1).broadcast_to([128, 16, DE]),
                        ).then_inc(s_pl, 1)
                    if u >= 12 and (u - 12) % 4 == 0 and (u - 12) // 4 < ng:
                        g = (u - 12) // 4
                        pool.wait_ge(s_act, done[("final", 4 * g + 3)])
                        half = 4096 * ((4 * g) % DO // 4)
                        pool.dma_start(
                            out=out_dst(g), in_=outsb[:, half : half + 4096]
                        ).then_inc(s_od[g % 2], 16)

            @block.tensor
            def _(tensor):
                tensor.wait_ge(s_c, CONST_TARGET)
                for u in range(nt + 8):
                    tT = u - 5
                    tM = u - 7
                    doT = 0 <= tT < nt
                    doM = 0 <= tM < nt
                    inc = done[("peiter", u)]
                    if doT:
                        tensor.wait_ge(s_pl, done[("prescale", tT)])
                        if tT >= DP1:
                            tensor.wait_ge(s_dve, done[("copy1", tT - DP1)])
                        for c in range(8):
                            mi = nc.tensor.transpose(
                                ps1[tT % DP1][:, 128 * c : 128 * c + 128],
                                xs[tT % DXS][:, 128 * c : 128 * c + 128],
                                idb[:],
                            )
                            if c == 7 and not doM:
                                mi.then_inc(s_pe, inc)
                    if doM:
                        if tM >= DP2:
                            tensor.wait_ge(s_act, done[("final", tM - DP2)])
                        pt = ps2[tM % DP2]
                        rp = (
                            rpm[:, 64 * tM : 64 * tM + 64]
                            .unsqueeze(1)
                            .broadcast_to([128, 8, DE])
                        )
                        cpm = cpm0 if tM < nt // 2 else cpm1
                        for h in range(2):
                            nc.tensor.matmul(
                                pt[:, 512 * h : 512 * h + 512], i4[:], rp,
                                start=True, stop=False, skip_group_check=True,
                            )
                        for h in range(2):
                            nc.tensor.matmul(
                                pt[:, 512 * h : 512 * h + 512],
                                i32[:],
                                cpm[:, 512 * h : 512 * h + 512],
                                start=False, stop=False, skip_group_check=True,
                            )
                        tensor.wait_ge(s_dve, done[("copy1", tM)])
                        for c in range(8):
                            mi = nc.tensor.matmul(
                                pt[:, 128 * c : 128 * c + 128],
                                xTs[tM % DXT][:, 128 * c : 128 * c + 128],
                                wg[:],
                                start=False, stop=True, skip_group_check=True,
                            )
                            if c == 7:
                                mi.then_inc(s_pe, inc)

    return nc


_NC_CACHE = {}


def _get_nc():
    if "nc" not in _NC_CACHE:
        _NC_CACHE["nc"] = _build_nc()
    return _NC_CACHE["nc"]


def _make_in_maps(edges, nodes, g_node, g_edge, W, b):
    import ml_dtypes

    bf = ml_dtypes.bfloat16
    edges = np.ascontiguousarray(edges, dtype=np.float32)
    nodes = np.ascontiguousarray(nodes, dtype=np.float32)

    # ---- host: tiny node path (B*N*dn = 131K elems)
    ms = np.mean(np.square(nodes), axis=-1, keepdims=True)
    nodes_n = nodes / np.sqrt(ms + EPS) * g_node  # [B, N, 128]
    Wr, Wc, We = W[:DN], W[DN : 2 * DN], W[2 * DN :]
    row_proj = (nodes_n @ Wr).astype(np.float32)  # [B, N, 64]
    col_proj = (nodes_n @ Wc).astype(np.float32)  # [B, N, 64]
    Wg = (g_edge[:, None] * We).astype(np.float32)  # fold g_edge into We

    # block-diagonal Wg (even-r rows top-left, odd-r rows bottom-right)
    wgblk = np.zeros((128, 128), dtype=np.float32)
    wgblk[:64, :64] = Wg
    wgblk[64:, 64:] = Wg
    wgblk = wgblk.astype(bf)
    idb = np.eye(128, dtype=np.float32).astype(bf)
    i4 = np.zeros((128, 128), np.float32)
    i4[:4] = np.kron(np.eye(4, dtype=np.float32), np.ones((1, 32), np.float32))
    i4 = i4.astype(bf)
    i32 = np.zeros((128, 128), np.float32)
    i32[:32] = np.tile(np.eye(32, dtype=np.float32), (1, 4))
    i32 = i32.astype(bf)

    # col_proj + bias, per b: cpm_b[p', 64r + e'] = cp_b[16 p' + r, e']
    cp = (col_proj + b).astype(np.float32)  # [B, 512, 64]
    cpm0 = np.zeros((128, 1024), np.float32)
    cpm0[:32] = cp[0].reshape(32, 1024)
    cpm0 = cpm0.astype(bf)
    cpm1 = np.zeros((128, 1024), np.float32)
    cpm1[:32] = cp[1].reshape(32, 1024)
    cpm1 = cpm1.astype(bf)

    in_maps = []
    for c in range(NCORES):
        xc = edges[:, c * IPC : (c + 1) * IPC]  # [B, 64, 512, 64]
        xc = np.ascontiguousarray(xc).reshape(NBLK, N, DE)
        # rpm[k, 64T + e'] = row_proj[block 4T + k][e']
        rp = row_proj[:, c * IPC : (c + 1) * IPC].reshape(NBLK, DE)  # [128, 64]
        rpm = np.zeros((128, NT * DE), np.float32)
        rpm[:4] = rp.reshape(NT, 4, DE).transpose(1, 0, 2).reshape(4, NT * DE)
        rpm = rpm.astype(bf)
        in_maps.append(
            {
                "x": xc,
                "wg": wgblk,
                "idb": idb,
                "i4": i4,
                "i32": i32,
                "rpm": rpm,
                "cpm0": cpm0,
                "cpm1": cpm1,
            }
        )
    return in_maps


def _run(edges, nodes, g_node, g_edge, W, b, trace=False, **spmd_kwargs):
    in_maps = _make_in_maps(edges, nodes, g_node, g_edge, W, b)

    from concourse.bass_utils import run_bass_kernel_spmd

    nc = _get_nc()
    res = run_bass_kernel_spmd(
        nc, in_maps, list(range(NCORES)), trace=trace, **spmd_kwargs
    )

    out = np.empty((B, N, N, DE), dtype=np.float32)
    for c in range(NCORES):
        oc = res.results[c]["out"].reshape(B, IPC, N, DE)
        out[:, c * IPC : (c + 1) * IPC] = oc
    return out, res


def kernel(edges, nodes, g_node, g_edge, W, b):
    return _run(edges, nodes, g_node, g_edge, W, b)[0]


if __name__ == "__main__":
    rng = np.random.default_rng(0)
    edges = rng.standard_normal((B, N, N, DE), dtype=np.float32)
    nodes = rng.standard_normal((B, N, DN), dtype=np.float32)
    g_node = np.ones(DN, np.float32)
    g_edge = np.ones(DE, np.float32)
    W = rng.standard_normal((2 * DN + DE, DE), dtype=np.float32) / 18.0
    b = (rng.standard_normal(DE) * 0.01).astype(np.float32)
    o = kernel(edges, nodes, g_node, g_edge, W, b)
    print(o.shape, o.dtype)
